# revision 1
# baseline (speedup 1.0000x reference)
"""Trainium2 Bass kernel for nn_DecodeMoeOps (MoE decode: dispatch-quant,
grouped int8 GEMM1, SwiGLU, requant, grouped int8 GEMM2, weighted combine).

Expert-parallel across 8 NeuronCores: core c owns experts {2c, 2c+1} and
computes, for ALL 128 tokens, its 2 experts' contributions weighted by the
combine matrix; the host sums the 8 partial outputs. Combine weights are zero
for unrouted (token, expert) pairs, so this matches the reference's dense
compute exactly.
"""

import os
import sys

for _p in ("/opt/trn_rl_repo", "/root/.axon_site/_ro/trn_rl_repo"):
    if os.path.isdir(_p) and _p not in sys.path:
        sys.path.insert(0, _p)

from contextlib import ExitStack

import ml_dtypes
import numpy as np

import concourse.bass as bass
import concourse.mybir as mybir
import concourse.tile as tile
from concourse import bacc
from concourse.bass_utils import run_bass_kernel_spmd
from concourse.masks import make_identity

B, TOPK, H, I, E = 128, 8, 2048, 1408, 16
NCORES = 8
EPC = E // NCORES  # experts per core
KH = H // 128  # 16 k-tiles for GEMM1 contraction
KI = I // 128  # 11 k-tiles for GEMM2 contraction
I2 = 2 * I
F32 = mybir.dt.float32
BF16 = mybir.dt.bfloat16
MAGIC = float(3 * 2**22)  # 1.5*2^23: fp32 round-to-int magic (covers negatives)

# chunking of a 1408-wide GEMM1 half across PSUM (bank = 512 fp32)
N1_CHUNKS = [(0, 512), (512, 512), (1024, 384)]
N2_CHUNKS = [(0, 512), (512, 512), (1024, 512), (1536, 512)]

# weight storage in HBM: "int8" ships 1 B/weight and casts to bf16 during the
# SWDGE DMA; "bf16" ships 2 B/weight over HWDGE with no cast.
VARIANT = os.environ.get("MOE_VARIANT", "int8")
KG1 = 4  # w1 k-tiles per consolidated DMA (16 = 4 groups of 4)
W2_GROUPS = [(0, 2), (2, 2), (4, 2), (6, 2), (8, 2), (10, 1)]  # w2 DMA groups

_cache: dict = {}


def _build_program(debug_taps=False):
    nc = bacc.Bacc(
        "TRN2",
        target_bir_lowering=False,
        debug=False,
        num_devices=NCORES,
    )
    mult = mybir.AluOpType.mult

    WDT = mybir.dt.int8 if VARIANT == "int8" else BF16

    # --- per-core DRAM I/O ---
    xqT_d = nc.dram_tensor("xqT", [128, H], BF16, kind="ExternalInput").ap()
    sx_d = nc.dram_tensor("sx", [128, 1], F32, kind="ExternalInput").ap()
    comb_d = nc.dram_tensor("combs", [128, EPC], F32, kind="ExternalInput").ap()
    # w1 tiled [expert, half(gate/up), k, p, f]
    w1_d = nc.dram_tensor(
        "w1t", [EPC, 2, KH, 128, I], WDT, kind="ExternalInput"
    ).ap()
    # w2 tiled [expert, k, p, f]
    w2_d = nc.dram_tensor("w2t", [EPC, KI, 128, H], WDT, kind="ExternalInput").ap()
    sc1_d = nc.dram_tensor("scale1", [EPC, I2], F32, kind="ExternalInput").ap()
    sc2_d = nc.dram_tensor("scale2", [EPC, H], F32, kind="ExternalInput").ap()
    y_d = nc.dram_tensor("y", [128, H], F32, kind="ExternalOutput").ap()
    taps = {}
    if debug_taps:
        for e in range(EPC):
            taps[f"dbg_deq0_{e}"] = nc.dram_tensor(f"dbg_deq0_{e}", [128, I], F32, kind="ExternalOutput").ap()
            taps[f"dbg_deq1_{e}"] = nc.dram_tensor(f"dbg_deq1_{e}", [128, I], F32, kind="ExternalOutput").ap()
            taps[f"dbg_act_{e}"] = nc.dram_tensor(f"dbg_act_{e}", [128, I], F32, kind="ExternalOutput").ap()
            taps[f"dbg_mc_{e}"] = nc.dram_tensor(f"dbg_mc_{e}", [128, 1], F32, kind="ExternalOutput").ap()
            taps[f"dbg_aq_{e}"] = nc.dram_tensor(f"dbg_aq_{e}", [128, I], F32, kind="ExternalOutput").ap()
            taps[f"dbg_aqT_{e}"] = nc.dram_tensor(f"dbg_aqT_{e}", [128, KI * 128], F32, kind="ExternalOutput").ap()

    with tile.TileContext(nc) as tc, ExitStack() as ctx:
        consts = ctx.enter_context(tc.tile_pool(name="consts", bufs=1))
        rows = ctx.enter_context(tc.tile_pool(name="rows", bufs=1))
        bcast = ctx.enter_context(tc.tile_pool(name="bcast", bufs=1))
        w1_pool = ctx.enter_context(tc.tile_pool(name="w1p", bufs=4))
        w2_pool = ctx.enter_context(tc.tile_pool(name="w2p", bufs=6))
        actp = ctx.enter_context(tc.tile_pool(name="actp", bufs=1))
        aqTp = ctx.enter_context(tc.tile_pool(name="aqTp", bufs=2))
        w2sp = ctx.enter_context(tc.tile_pool(name="w2sp", bufs=2))
        stats = ctx.enter_context(tc.tile_pool(name="stats", bufs=2))
        yp = ctx.enter_context(tc.tile_pool(name="yp", bufs=1))
        ychunkp = ctx.enter_context(tc.tile_pool(name="ychunkp", bufs=4))
        ps1_pool = ctx.enter_context(tc.tile_pool(name="ps1", bufs=1, space="PSUM"))
        ps2_pool = ctx.enter_context(tc.tile_pool(name="ps2", bufs=1, space="PSUM"))
        psT_pool = ctx.enter_context(tc.tile_pool(name="psT", bufs=1, space="PSUM"))

        # --- prologue: constants ---
        xqT_s = consts.tile([128, H], BF16, name="xqT_s")
        nc.sync.dma_start(out=xqT_s[:], in_=xqT_d)
        ident = consts.tile([128, 128], BF16, name="ident")
        make_identity(nc, ident[:])
        sx_s = consts.tile([128, 1], F32, name="sx_s")
        nc.sync.dma_start(out=sx_s[:], in_=sx_d)
        comb_s = consts.tile([128, EPC], F32, name="comb_s")
        nc.sync.dma_start(out=comb_s[:], in_=comb_d)
        ones_s = consts.tile([1, 128], F32, name="ones_s")
        nc.vector.memset(ones_s[:], 1.0)

        def bcast_row(row_ap, dst, width, ename):
            # dst[p, f] = row_ap[0, f] via PE outer product ones^T @ row
            for off in range(0, width, 512):
                sz = min(512, width - off)
                psc = psT_pool.tile([128, 512], F32, tag="psT", name=f"psb_{ename}_{off}")
                nc.tensor.matmul(
                    psc[:, 0:sz],
                    lhsT=ones_s[:],
                    rhs=row_ap[0:1, off : off + sz],
                    start=True,
                    stop=True,
                )
                nc.vector.tensor_copy(out=dst[:, off : off + sz], in_=psc[:, 0:sz])

        # Phase A per expert: GEMM1 + SwiGLU + requant + transpose -> aqT.
        # Phase B per expert: GEMM2 + dequant + y. DMA stream order is
        # w1(e0), w1(e1), w2(e0), w2(e1): the last expert's epilogue chain
        # finishes while w2 still streams, so the kernel tail is only the
        # final chunk's dequant.
        contribs = []
        aqTs, s2cs, W2Ss = [], [], []
        for e in range(EPC):
            # --- per-channel dequant scales, broadcast across partitions ---
            row1 = rows.tile([1, I2], F32, tag="row1", name=f"row1_{e}")
            nc.sync.dma_start(out=row1[:], in_=sc1_d[e : e + 1, :])
            S1 = bcast.tile([128, I2], F32, tag="S1", name=f"S1_{e}")
            bcast_row(row1, S1, I2, f"s1_{e}")
            row2 = rows.tile([1, H], F32, tag="row2", name=f"row2_{e}")
            nc.sync.dma_start(out=row2[:], in_=sc2_d[e : e + 1, :])
            W2S = w2sp.tile([128, H], F32, tag="W2S", name=f"W2S_{e}")
            bcast_row(row2, W2S, H, f"w2s_{e}")
            W2Ss.append(W2S)

            # --- GEMM1: h = xq @ w1[e]  (token-stationary; k-outer) ---
            deqs = []
            for half in range(2):
                ps1 = ps1_pool.tile([128, I], F32, tag="ps1", name=f"ps1_{e}_{half}")
                for g in range(KH // KG1):
                    w1s = w1_pool.tile(
                        [128, KG1, I], BF16, tag="w1s", name=f"w1s_{e}_{half}_{g}"
                    )
                    src = w1_d[e, half, g * KG1 : (g + 1) * KG1].rearrange(
                        "j p f -> p j f"
                    )
                    if VARIANT == "int8":
                        nc.gpsimd.dma_start(out=w1s[:], in_=src)
                    else:
                        nc.sync.dma_start(out=w1s[:], in_=src)
                    for j in range(KG1):
                        k = g * KG1 + j
                        for off, sz in N1_CHUNKS:
                            nc.tensor.matmul(
                                ps1[:, off : off + sz],
                                lhsT=xqT_s[:, k * 128 : (k + 1) * 128],
                                rhs=w1s[:, j, off : off + sz],
                                start=(k == 0),
                                stop=(k == KH - 1),
                            )
                # dequant: (psum * sx[b]) * S1[chan]
                deq = actp.tile([128, I], F32, tag=f"deq{half}", name=f"deq_{e}_{half}")
                nc.vector.scalar_tensor_tensor(
                    out=deq[:],
                    in0=ps1[:],
                    scalar=sx_s[:, 0:1],
                    in1=S1[:, half * I : (half + 1) * I],
                    op0=mult,
                    op1=mult,
                )
                deqs.append(deq)
            gate_deq, up_deq = deqs

            # --- SwiGLU: act = gate * sigmoid(gate) * up (smooth folded into up scale) ---
            sig = actp.tile([128, I], F32, tag="sig", name=f"sig_{e}")
            nc.scalar.activation(
                out=sig[:], in_=gate_deq[:], func=mybir.ActivationFunctionType.Sigmoid
            )
            gsig = actp.tile([128, I], F32, tag="gsig", name=f"gsig_{e}")
            nc.vector.tensor_tensor(out=gsig[:], in0=gate_deq[:], in1=sig[:], op=mult)
            act = actp.tile([128, I], F32, tag="sig", name=f"act_{e}")
            nc.vector.tensor_tensor(out=act[:], in0=gsig[:], in1=up_deq[:], op=mult)

            # --- dynamic requant: aq = round(act * 127 / max|act|) ---
            m = stats.tile([128, 1], F32, tag="m", name=f"m_{e}")
            nc.vector.reduce_max(
                out=m[:], in_=act[:], axis=mybir.AxisListType.X,
                apply_absolute_value=True,
            )
            mc = stats.tile([128, 1], F32, tag="mc", name=f"mc_{e}")
            nc.vector.tensor_scalar_max(out=mc[:], in0=m[:], scalar1=1e-12)
            r = stats.tile([128, 1], F32, tag="r", name=f"r_{e}")
            nc.vector.reciprocal(out=r[:], in_=mc[:])
            r127 = stats.tile([128, 1], F32, tag="r127", name=f"r127_{e}")
            nc.vector.tensor_scalar_mul(out=r127[:], in0=r[:], scalar1=127.0)
            # s2c = (mc/127) * comb[:, e]
            s2c = stats.tile([128, 1], F32, tag="s2c", name=f"s2c_{e}")
            nc.vector.scalar_tensor_tensor(
                out=s2c[:],
                in0=mc[:],
                scalar=1.0 / 127.0,
                in1=comb_s[:, e : e + 1],
                op0=mult,
                op1=mult,
            )
            # magic-constant round-to-nearest-even, output exact ints in bf16
            t = actp.tile([128, I], F32, tag="deq0", name=f"t_{e}")
            nc.scalar.activation(
                out=t[:],
                in_=act[:],
                func=mybir.ActivationFunctionType.Copy,
                bias=MAGIC,
                scale=r127[:, 0:1],
            )
            aq = actp.tile([128, I], BF16, tag="aq", name=f"aq_{e}")
            nc.vector.tensor_scalar_add(out=aq[:], in0=t[:], scalar1=-MAGIC)

            # --- transpose aq -> aqT (I on partitions) via PE transpose ---
            aqT = aqTp.tile([128, KI * 128], BF16, tag="aqT", name=f"aqT_{e}")
            for k in range(KI):
                psT = psT_pool.tile([128, 128], BF16, tag="psT", name=f"psT_{e}_{k}")
                nc.tensor.transpose(
                    psT[:], aq[:, k * 128 : (k + 1) * 128], ident[:]
                )
                nc.vector.tensor_copy(
                    out=aqT[:, k * 128 : (k + 1) * 128], in_=psT[:]
                )
            aqTs.append(aqT)
            s2cs.append(s2c)
            if debug_taps:
                nc.gpsimd.dma_start(out=taps[f"dbg_deq0_{e}"], in_=gate_deq[:])
                nc.gpsimd.dma_start(out=taps[f"dbg_deq1_{e}"], in_=up_deq[:])
                nc.gpsimd.dma_start(out=taps[f"dbg_act_{e}"], in_=act[:])
                nc.gpsimd.dma_start(out=taps[f"dbg_mc_{e}"], in_=mc[:])
                nc.gpsimd.dma_start(out=taps[f"dbg_aq_{e}"], in_=aq[:])
                nc.gpsimd.dma_start(out=taps[f"dbg_aqT_{e}"], in_=aqT[:])

        for e in range(EPC):
            aqT, s2c, W2S = aqTs[e], s2cs[e], W2Ss[e]
            # --- GEMM2: o = aq @ w2[e] ---
            # k-outer so each streamed w2 tile is consumed on arrival; the
            # LAST k-group runs chunk-major so per-chunk dequant + y output
            # pipeline into the final matmuls (short kernel tail).
            if e == 0:
                contrib0 = yp.tile([128, H], F32, name="contrib0")
                contribs.append(contrib0)
            ps2c = {
                off: ps2_pool.tile(
                    [128, 512], F32, tag=f"ps2_{off}", name=f"ps2_{e}_{off}"
                )
                for off, _ in N2_CHUNKS
            }
            n_groups = len(W2_GROUPS)
            for g, (g0, gn) in enumerate(W2_GROUPS):
                w2s = w2_pool.tile([128, 2, H], BF16, tag="w2s", name=f"w2s_{e}_{g0}")
                src = w2_d[e, g0 : g0 + gn].rearrange("j p f -> p j f")
                if VARIANT == "int8":
                    nc.gpsimd.dma_start(out=w2s[:, 0:gn, :], in_=src)
                else:
                    nc.sync.dma_start(out=w2s[:, 0:gn, :], in_=src)
                last_group = g == n_groups - 1

                def mm2(j, off, sz):
                    k = g0 + j
                    nc.tensor.matmul(
                        ps2c[off][:, 0:sz],
                        lhsT=aqT[:, k * 128 : (k + 1) * 128],
                        rhs=w2s[:, j, off : off + sz],
                        start=(k == 0),
                        stop=(k == KI - 1),
                    )

                if not last_group:
                    for j in range(gn):
                        for off, sz in N2_CHUNKS:
                            mm2(j, off, sz)
                else:
                    # all but the final k-tile in normal (k-minor) order
                    for j in range(gn - 1):
                        for off, sz in N2_CHUNKS:
                            mm2(j, off, sz)
                    # final k-tile chunk-major: each chunk completes in turn
                    # (single LDWEIGHTS: lhsT is fixed at k = KI-1)
                    for off, sz in N2_CHUNKS:
                        mm2(gn - 1, off, sz)
                        # chunk complete: dequant + combine weight
                        if e == 0:
                            nc.vector.scalar_tensor_tensor(
                                out=contrib0[:, off : off + sz],
                                in0=ps2c[off][:, 0:sz],
                                scalar=s2c[:, 0:1],
                                in1=W2S[:, off : off + sz],
                                op0=mult,
                                op1=mult,
                            )
                        else:
                            tmpc = ychunkp.tile(
                                [128, 512], F32, tag="tmpc", name=f"tmpc_{off}"
                            )
                            nc.vector.scalar_tensor_tensor(
                                out=tmpc[:, 0:sz],
                                in0=ps2c[off][:, 0:sz],
                                scalar=s2c[:, 0:1],
                                in1=W2S[:, off : off + sz],
                                op0=mult,
                                op1=mult,
                            )
                            youtc = ychunkp.tile(
                                [128, 512], F32, tag="youtc", name=f"youtc_{off}"
                            )
                            nc.vector.tensor_tensor(
                                out=youtc[:, 0:sz],
                                in0=contribs[0][:, off : off + sz],
                                in1=tmpc[:, 0:sz],
                                op=mybir.AluOpType.add,
                            )
                            nc.sync.dma_start(
                                out=y_d[:, off : off + sz], in_=youtc[:, 0:sz]
                            )

    nc.compile()
    return nc


def get_program(debug_taps=False):
    key = ("nc", debug_taps)
    if key not in _cache:
        _cache[key] = _build_program(debug_taps=debug_taps)
    return _cache[key]


def _prep_inputs(x, expert_ids, smooth_scales, expert_scales, w1, w1_scale, w2, w2_scale):
    """Host-side dispatch: quantize x, build combine matrix, shard experts."""
    x = np.asarray(x, np.float32)
    expert_ids = np.asarray(expert_ids)
    smooth_scales = np.asarray(smooth_scales, np.float32)
    expert_scales = np.asarray(expert_scales, np.float32)
    w1_scale = np.asarray(w1_scale, np.float32)
    w2_scale = np.asarray(w2_scale, np.float32)

    # dynamic per-token int8 quantization (exact mirror of reference ops)
    sx = np.maximum(np.max(np.abs(x), axis=-1, keepdims=True), 1e-12) / 127.0
    xq = np.round(np.clip(x / sx, -128.0, 127.0)).astype(np.float32)  # ints

    # xqT tiled [p, k*128 + b] = xq[b, k*128 + p]
    xqT = np.ascontiguousarray(xq.T)  # [H, B]
    xqT_t = np.ascontiguousarray(
        xqT.reshape(KH, 128, B).transpose(1, 0, 2).reshape(128, KH * B)
    ).astype(ml_dtypes.bfloat16)

    # combine matrix [B, E]: scatter-add expert_scales at expert_ids
    comb = np.zeros((B, E), np.float32)
    np.add.at(comb, (np.arange(B)[:, None], expert_ids), expert_scales)

    w1v = w1.astype(np.int8)  # int8-valued
    w2v = w2.astype(np.int8)
    wdt = np.int8 if VARIANT == "int8" else ml_dtypes.bfloat16

    in_maps = []
    for c in range(NCORES):
        es = list(range(c * EPC, (c + 1) * EPC))
        # w1 [e, H, 2I] -> [e, half, k, p, f] bf16
        w1c = w1v[es].reshape(EPC, KH, 128, I2)
        w1gu = np.stack([w1c[..., :I], w1c[..., I:]], axis=1)  # [e,2,k,p,I]
        w1_bf = np.ascontiguousarray(w1gu).astype(wdt)
        w2_bf = np.ascontiguousarray(
            w2v[es].reshape(EPC, KI, 128, H)
        ).astype(wdt)
        # dequant scale rows; smooth folded into the up half
        sc1 = np.concatenate(
            [w1_scale[es][:, :I], w1_scale[es][:, I:] * smooth_scales[es]], axis=1
        ).astype(np.float32)
        sc2 = w2_scale[es].astype(np.float32)
        in_maps.append(
            {
                "xqT": xqT_t,
                "sx": sx.astype(np.float32),
                "combs": np.ascontiguousarray(comb[:, es]).astype(np.float32),
                "w1t": w1_bf,
                "w2t": w2_bf,
                "scale1": sc1,
                "scale2": sc2,
            }
        )
    return in_maps


def kernel(
    x,
    expert_ids,
    smooth_scales,
    expert_scales,
    x_active_mask,
    w1,
    w1_scale,
    w2,
    w2_scale,
    _trace=False,
    _trace_kwargs=None,
):
    in_maps = _prep_inputs(
        x, expert_ids, smooth_scales, expert_scales, w1, w1_scale, w2, w2_scale
    )
    nc = get_program()
    res = run_bass_kernel_spmd(
        nc,
        in_maps,
        core_ids=list(range(NCORES)),
        trace=_trace,
        **(_trace_kwargs or {}),
    )
    y = np.zeros((B, H), np.float32)
    for r in res.results:
        y += r["y"]
    y *= np.asarray(x_active_mask).astype(np.float32)[:, None]
    if _trace:
        kernel.last_results = res
    return y



# revision 19
# speedup vs baseline: 1.4823x; 1.4823x over previous
"""Trainium2 Bass kernel for nn_DecodeMoeOps (MoE decode: dispatch-quant,
grouped int8 GEMM1, SwiGLU, requant, grouped int8 GEMM2, weighted combine).

Expert-parallel across 8 NeuronCores: core c owns experts {2c, 2c+1}. Each
core computes only the tokens routed to its experts (gathered host-side,
padded to N_PAD), using weight-stationary GEMMs over routed tokens:

  GEMM1: out[f, tok] = w1_tile[k,f].T @ xqs[k, tok]   (xqs = fp16(xq*sx))
  GEMM2: out[h, tok] = w2_tile[i,h].T @ aq[i, tok]

w1 ships as int8 and is cast to fp16 on-chip (split across DVE/ACT/GPSIMD);
w2 ships as fp8e3m4 (exact for |w|<=31, max abs err 2 above) with the 16x
scale folded into w2_scale. Per-channel dequant scales are per-partition in
this layout; the requant absmax runs on GPSIMD partition_all_reduce. Host
scatters the per-expert [h, tok] outputs back into y[B, H].
"""

import os
import sys

for _p in ("/opt/trn_rl_repo", "/root/.axon_site/_ro/trn_rl_repo"):
    if os.path.isdir(_p) and _p not in sys.path:
        sys.path.insert(0, _p)

from contextlib import ExitStack

import ml_dtypes
import numpy as np

import concourse.bass as bass
import concourse.mybir as mybir
import concourse.tile as tile
from concourse import bacc
from concourse import bass_isa
from concourse.bass_utils import run_bass_kernel_spmd

B, TOPK, H, I, E = 128, 8, 2048, 1408, 16
NCORES = 8
EPC = E // NCORES  # experts per core
KH = H // 128  # 16 k-tiles for GEMM1 contraction
KI = I // 128  # 11 k-tiles for GEMM2 contraction
FT = I // 128  # 11 f-tiles per GEMM1 half
HT = H // 128  # 16 h-tiles for GEMM2 output
I2 = 2 * I
F32 = mybir.dt.float32
BF16 = mybir.dt.bfloat16
F16 = mybir.dt.float16
I8 = mybir.dt.int8
F8E3 = mybir.dt.float8e3
MAGIC = float(3 * 2**22)  # fp32 round-to-int magic (covers negatives)

# on-chip int8->fp16 cast: free-dim split of each [128, 2816] w1 k-tile
CAST_DVE = (0, 1536)
CAST_ACT = (1536, 768)
CAST_POOL = (2304, 512)

_cache: dict = {}


def _build_program(n_pad: int):
    mult = mybir.AluOpType.mult
    nc = bacc.Bacc(
        "TRN2",
        target_bir_lowering=False,
        debug=False,
        num_devices=NCORES,
    )

    FW = FT * n_pad   # gate/up accumulator width
    HW = HT * n_pad   # GEMM2 accumulator width

    # --- per-core DRAM I/O ---
    xqsT_d = nc.dram_tensor("xqsT", [EPC, 128, KH, n_pad], F16, kind="ExternalInput").ap()
    w1_d = nc.dram_tensor("w1t", [EPC, KH, 128, I2], I8, kind="ExternalInput").ap()
    w2_d = nc.dram_tensor("w2t", [EPC, KI, 128, H], F8E3, kind="ExternalInput").ap()
    s1g_d = nc.dram_tensor("s1g", [EPC, 128, FT], F32, kind="ExternalInput").ap()
    s1u_d = nc.dram_tensor("s1u", [EPC, 128, FT], F32, kind="ExternalInput").ap()
    w2s_d = nc.dram_tensor("scale2", [EPC, 128, HT], F32, kind="ExternalInput").ap()
    comb_d = nc.dram_tensor("combs", [EPC, 128, n_pad], F32, kind="ExternalInput").ap()
    o_d = nc.dram_tensor("o", [EPC, 128, HW], F32, kind="ExternalOutput").ap()

    with tile.TileContext(nc) as tc, ExitStack() as ctx:
        consts = ctx.enter_context(tc.tile_pool(name="consts", bufs=1))
        w1i8p = ctx.enter_context(tc.tile_pool(name="w1i8", bufs=5))
        w1f16p = ctx.enter_context(tc.tile_pool(name="w1f16", bufs=5))
        w2p = ctx.enter_context(tc.tile_pool(name="w2p", bufs=1))
        epi = ctx.enter_context(tc.tile_pool(name="epi", bufs=2))
        op_ = ctx.enter_context(tc.tile_pool(name="op", bufs=2))
        ps1_pool = ctx.enter_context(tc.tile_pool(name="ps1", bufs=2, space="PSUM"))
        ps2_pool = ctx.enter_context(tc.tile_pool(name="ps2", bufs=1, space="PSUM"))

        # --- prologue: small inputs ---
        xqs_s = consts.tile([128, EPC, KH, n_pad], F16, name="xqs_s")
        nc.scalar.dma_start(out=xqs_s[:], in_=xqsT_d.rearrange("e p k j -> p e k j"))
        s1g_s = consts.tile([128, EPC, FT], F32, name="s1g_s")
        nc.scalar.dma_start(out=s1g_s[:], in_=s1g_d.rearrange("e p t -> p e t"))
        s1u_s = consts.tile([128, EPC, FT], F32, name="s1u_s")
        nc.scalar.dma_start(out=s1u_s[:], in_=s1u_d.rearrange("e p t -> p e t"))
        w2s_s = consts.tile([128, EPC, HT], F32, name="w2s_s")
        nc.scalar.dma_start(out=w2s_s[:], in_=w2s_d.rearrange("e p t -> p e t"))
        comb_s = consts.tile([128, EPC, n_pad], F32, name="comb_s")
        nc.scalar.dma_start(out=comb_s[:], in_=comb_d.rearrange("e p j -> p e j"))

        def bank_flags(offsets_bytes):
            """PSUM accumulation start/stop flags per chunk: matmul start=True
            zeroes the whole 2KB bank, so exactly one start (first chunk) and
            one stop (last chunk) per bank. Offsets must not cross banks."""
            first, last = {}, {}
            for i, off in enumerate(offsets_bytes):
                b = off // 2048
                if b not in first:
                    first[b] = i
                last[b] = i
            starts = {i for i in first.values()}
            stops = {i for i in last.values()}
            return starts, stops

        def epilogue1(e, ps1_e):
            """dequant + SwiGLU + requant -> aq [128, FT, n_pad]."""
            ps_g = ps1_e[:, 0:FW]
            ps_u = ps1_e[:, FW : 2 * FW]
            s1g_b = epi.tile([128, FT, n_pad], F32, tag="s1gb", name=f"s1gb_{e}")
            nc.vector.tensor_copy(
                out=s1g_b[:],
                in_=s1g_s[:, e, :].unsqueeze(2).broadcast_to([128, FT, n_pad]),
            )
            s1u_b = epi.tile([128, FT, n_pad], F32, tag="s1ub", name=f"s1ub_{e}")
            nc.vector.tensor_copy(
                out=s1u_b[:],
                in_=s1u_s[:, e, :].unsqueeze(2).broadcast_to([128, FT, n_pad]),
            )
            gate = epi.tile([128, FW], F32, tag="gate", name=f"gate_{e}")
            nc.vector.tensor_tensor(
                out=gate[:], in0=ps_g, in1=s1g_b[:].rearrange("p t n -> p (t n)"), op=mult
            )
            sig = epi.tile([128, FW], F32, tag="sig", name=f"sig_{e}")
            nc.scalar.activation(
                out=sig[:], in_=gate[:], func=mybir.ActivationFunctionType.Sigmoid
            )
            # gdu = gate * s1u (GPSIMD has no PSUM port; reads gate from SBUF)
            gdu = epi.tile([128, FW], F32, tag="gdu", name=f"gdu_{e}")
            nc.vector.tensor_tensor(
                out=gdu[:], in0=gate[:], in1=s1u_b[:].rearrange("p t n -> p (t n)"), op=mult
            )
            t1 = epi.tile([128, FW], F32, tag="t1", name=f"t1_{e}")
            nc.gpsimd.tensor_tensor(out=t1[:], in0=gdu[:], in1=sig[:], op=mult)
            act2 = epi.tile([128, FW], F32, tag="act2", name=f"act2_{e}")
            nc.vector.tensor_tensor(out=act2[:], in0=t1[:], in1=ps_u, op=mult)

            am = epi.tile([128, FW], F32, tag="am", name=f"am_{e}")
            nc.gpsimd.partition_all_reduce(
                am[:], act2[:], channels=128, reduce_op=bass_isa.ReduceOp.absmax
            )
            m = epi.tile([128, n_pad], F32, tag="m", name=f"m_{e}")
            nc.vector.tensor_reduce(
                out=m[:],
                in_=am[:].rearrange("p (t n) -> p n t", t=FT),
                op=mybir.AluOpType.max,
                axis=mybir.AxisListType.X,
            )
            mc = epi.tile([128, n_pad], F32, tag="mc", name=f"mc_{e}")
            nc.vector.tensor_scalar_max(out=mc[:], in0=m[:], scalar1=1e-12)
            r = epi.tile([128, n_pad], F32, tag="r", name=f"r_{e}")
            nc.vector.reciprocal(out=r[:], in_=mc[:])
            tq = epi.tile([128, FW], F32, tag="tq", name=f"tq_{e}")
            nc.vector.scalar_tensor_tensor(
                out=tq[:].rearrange("p (t n) -> p t n", t=FT),
                in0=act2[:].rearrange("p (t n) -> p t n", t=FT),
                scalar=127.0,
                in1=r[:].unsqueeze(1).broadcast_to([128, FT, n_pad]),
                op0=mult,
                op1=mult,
            )
            tq2 = epi.tile([128, FW], F32, tag="tq2", name=f"tq2_{e}")
            nc.scalar.activation(
                out=tq2[:], in_=tq[:], func=mybir.ActivationFunctionType.Copy, bias=MAGIC
            )
            aq = epi.tile([128, FT, n_pad], BF16, tag="aq", name=f"aq_{e}")
            nc.scalar.activation(
                out=aq[:].rearrange("p t n -> p (t n)"), in_=tq2[:],
                func=mybir.ActivationFunctionType.Copy, bias=-MAGIC,
            )
            # s2c = (mc/127)*comb and the combined GEMM2 dequant scale
            s2c = epi.tile([128, n_pad], F32, tag="s2c", name=f"s2c_{e}")
            nc.vector.scalar_tensor_tensor(
                out=s2c[:], in0=mc[:], scalar=1.0 / 127.0, in1=comb_s[:, e, :],
                op0=mult, op1=mult,
            )
            w2sc = epi.tile([128, HT, n_pad], F32, tag="w2sc", name=f"w2sc_{e}")
            nc.gpsimd.tensor_tensor(
                out=w2sc[:],
                in0=w2s_s[:, e, :].unsqueeze(2).broadcast_to([128, HT, n_pad]),
                in1=s2c[:].unsqueeze(1).broadcast_to([128, HT, n_pad]),
                op=mult,
            )
            return aq, w2sc

        def gemm2(e, aq, w2sc, w2tiles):
            """weight-stationary GEMM2 + per-bank dequant + output DMA.
            PSUM can only be read once a bank's accumulation group stopped,
            so the last-ki MMs and the dequant proceed bank by bank."""
            ps2 = ps2_pool.tile([128, HW], F32, tag="ps2", name=f"ps2_{e}")
            o_sb = op_.tile([128, HW], F32, tag="o_sb", name=f"o_{e}")
            cpb = max(1, 2048 // (n_pad * 4))  # h-chunks per PSUM bank
            banks = [list(range(b, min(b + cpb, HT))) for b in range(0, HT, cpb)]
            for ki in range(KI):
                last = ki == KI - 1
                for bi, bchunks in enumerate(banks):
                    for t in bchunks:
                        nc.tensor.matmul(
                            ps2[:, t * n_pad : (t + 1) * n_pad],
                            lhsT=w2tiles[ki][:, t * 128 : (t + 1) * 128],
                            rhs=aq[:, ki, :],
                            start=(ki == 0 and t in g2_starts),
                            stop=(last and t in g2_stops),
                        )
                    if not last:
                        continue
                    lo = bchunks[0] * n_pad
                    hi = (bchunks[-1] + 1) * n_pad
                    if bi < len(banks) - 1:
                        # earlier banks: ACT copies psum out, GPSIMD scales
                        od = epi.tile([128, hi - lo], F32, tag="odeq", name=f"od_{e}_{bi}")
                        nc.scalar.activation(
                            out=od[:], in_=ps2[:, lo:hi],
                            func=mybir.ActivationFunctionType.Copy,
                        )
                        nc.gpsimd.tensor_tensor(
                            out=o_sb[:, lo:hi],
                            in0=od[:],
                            in1=w2sc[:, bchunks[0] : bchunks[-1] + 1, :].rearrange(
                                "p t n -> p (t n)"
                            ),
                            op=mult,
                        )
                    else:
                        # final bank: one DVE op for the shortest tail chain
                        nc.vector.tensor_tensor(
                            out=o_sb[:, lo:hi],
                            in0=ps2[:, lo:hi],
                            in1=w2sc[:, bchunks[0] : bchunks[-1] + 1, :].rearrange(
                                "p t n -> p (t n)"
                            ),
                            op=mult,
                        )
                    nc.scalar.dma_start(
                        out=o_d[e, :, lo:hi], in_=o_sb[:, lo:hi]
                    )

        # --- main pipeline: expert-sequential so epilogue(e0) hides under
        # --- expert 1's cast stream; w2(e1) is the last DMA (tail-paced)
        g1_chunks = [(h, t) for h in (0, 1) for t in range(FT)]
        g1_chunks_last = [(h, t) for h in (1, 0) for t in range(FT)]
        s_idx, _ = bank_flags([h * FW * 4 + t * n_pad * 4 for h, t in g1_chunks])
        _, e_idx = bank_flags([h * FW * 4 + t * n_pad * 4 for h, t in g1_chunks_last])
        g1_starts = {g1_chunks[i] for i in s_idx}
        g1_stops = {g1_chunks_last[i] for i in e_idx}
        g2_starts, g2_stops = bank_flags([t * n_pad * 4 for t in range(HT)])
        w2tiles = {}
        aq_w2sc = {}
        for e in range(EPC):
            ps1_e = ps1_pool.tile([128, 2 * FW], F32, tag="ps1", name=f"ps1_{e}")
            for k in range(KH):
                w1i8 = w1i8p.tile([128, I2], I8, tag="w1i8", name=f"w1i8_{e}_{k}")
                nc.sync.dma_start(out=w1i8[:], in_=w1_d[e, k])
                w1f = w1f16p.tile([128, I2], F16, tag="w1f", name=f"w1f_{e}_{k}")
                o0, n0 = CAST_DVE
                nc.vector.tensor_copy(out=w1f[:, o0 : o0 + n0], in_=w1i8[:, o0 : o0 + n0])
                o1, n1 = CAST_ACT
                nc.scalar.activation(
                    out=w1f[:, o1 : o1 + n1],
                    in_=w1i8[:, o1 : o1 + n1],
                    func=mybir.ActivationFunctionType.Copy,
                )
                o2, n2 = CAST_POOL
                nc.gpsimd.tensor_copy(out=w1f[:, o2 : o2 + n2], in_=w1i8[:, o2 : o2 + n2])
                rhs = xqs_s[:, e, k, :]
                # at k=15 the up-half runs first so every bank's group has
                # stopped before the epilogue's first gate-region PSUM read
                chunks = g1_chunks if k < KH - 1 else g1_chunks_last
                for half, t in chunks:
                    base = half * FW
                    nc.tensor.matmul(
                        ps1_e[:, base + t * n_pad : base + (t + 1) * n_pad],
                        lhsT=w1f[:, half * I + t * 128 : half * I + (t + 1) * 128],
                        rhs=rhs,
                        start=(k == 0 and (half, t) in g1_starts),
                        stop=(k == KH - 1 and (half, t) in g1_stops),
                    )
            # w2(e) DMAs follow this expert's w1 stream in the SP queue
            w2tiles[e] = []
            for ki in range(KI):
                w2t = w2p.tile([128, H], F8E3, tag=f"w2_{e}_{ki}", name=f"w2_{e}_{ki}")
                nc.sync.dma_start(out=w2t[:], in_=w2_d[e, ki])
                w2tiles[e].append(w2t)
            aq_w2sc[e] = epilogue1(e, ps1_e)
            if e > 0:
                gemm2(e - 1, *aq_w2sc[e - 1], w2tiles[e - 1])
        gemm2(EPC - 1, *aq_w2sc[EPC - 1], w2tiles[EPC - 1])

    nc.compile()
    return nc


def get_program(n_pad: int):
    key = ("nc", n_pad)
    if key not in _cache:
        _cache[key] = _build_program(n_pad)
    return _cache[key]


def _routing(expert_ids, expert_scales):
    """comb[B, E] scatter-add; token lists per expert; N_PAD."""
    comb = np.zeros((B, E), np.float32)
    np.add.at(comb, (np.arange(B)[:, None], np.asarray(expert_ids)),
              np.asarray(expert_scales, np.float32))
    routed = np.zeros((B, E), bool)
    routed[np.arange(B)[:, None], np.asarray(expert_ids)] = True
    toks = [np.nonzero(routed[:, e])[0] for e in range(E)]
    max_n = max(len(t) for t in toks)
    n_pad = 16
    while n_pad < max_n:
        n_pad *= 2
    # PSUM chunking requires pow2 n_pad; >64 would overflow the 8 banks
    assert n_pad <= 64, f"routing too dense for this kernel: n_pad={n_pad}"
    return comb, toks, n_pad


def _prep_inputs(x, expert_ids, smooth_scales, expert_scales, w1, w1_scale, w2, w2_scale):
    """Host-side dispatch: quantize x, route tokens, shard experts."""
    x = np.asarray(x, np.float32)
    smooth_scales = np.asarray(smooth_scales, np.float32)
    w1_scale = np.asarray(w1_scale, np.float32)
    w2_scale = np.asarray(w2_scale, np.float32)

    # dynamic per-token int8 quantization (exact mirror of reference ops)
    sx = np.maximum(np.max(np.abs(x), axis=-1, keepdims=True), 1e-12) / 127.0
    xq = np.round(np.clip(x / sx, -128.0, 127.0)).astype(np.float32)
    xqs = (xq * sx).astype(np.float16)  # [B, H]
    xqsT = np.ascontiguousarray(
        xqs.T.reshape(KH, 128, B).transpose(1, 0, 2)
    )  # [128, KH, B]

    comb, toks, n_pad = _routing(expert_ids, expert_scales)

    w1v = np.asarray(w1).astype(np.int8)
    w2v = np.asarray(w2).astype(np.int8)

    in_maps = []
    for c in range(NCORES):
        es = list(range(c * EPC, (c + 1) * EPC))
        xqsT_e = np.zeros((EPC, 128, KH, n_pad), np.float16)
        comb_e = np.zeros((EPC, 128, n_pad), np.float32)
        for i, e in enumerate(es):
            tk = toks[e]
            xqsT_e[i, :, :, : len(tk)] = xqsT[:, :, tk]
            comb_e[i, :, : len(tk)] = comb[tk, e][None, :]
        w1c = w1v[es].reshape(EPC, KH, 128, I2)
        w2c = np.ascontiguousarray(
            (w2v[es].reshape(EPC, KI, 128, H).astype(np.float32) / 16.0)
        ).astype(ml_dtypes.float8_e3m4)
        # per-partition scale columns [e, p, T]
        s1g_full = w1_scale[es][:, :I]
        s1u_full = w1_scale[es][:, I:] * smooth_scales[es]
        s1g = np.ascontiguousarray(s1g_full.reshape(EPC, FT, 128).transpose(0, 2, 1))
        s1u = np.ascontiguousarray(s1u_full.reshape(EPC, FT, 128).transpose(0, 2, 1))
        sc2 = np.ascontiguousarray(
            (w2_scale[es] * 16.0).reshape(EPC, HT, 128).transpose(0, 2, 1))
        in_maps.append(
            {
                "xqsT": xqsT_e,
                "w1t": np.ascontiguousarray(w1c),
                "w2t": w2c,
                "s1g": s1g.astype(np.float32),
                "s1u": s1u.astype(np.float32),
                "scale2": sc2.astype(np.float32),
                "combs": comb_e,
            }
        )
    return in_maps, toks, n_pad


def kernel(
    x,
    expert_ids,
    smooth_scales,
    expert_scales,
    x_active_mask,
    w1,
    w1_scale,
    w2,
    w2_scale,
    _trace=False,
    _trace_kwargs=None,
):
    in_maps, toks, n_pad = _prep_inputs(
        x, expert_ids, smooth_scales, expert_scales, w1, w1_scale, w2, w2_scale
    )
    nc = get_program(n_pad)
    res = run_bass_kernel_spmd(
        nc,
        in_maps,
        core_ids=list(range(NCORES)),
        trace=_trace,
        **(_trace_kwargs or {}),
    )
    y = np.zeros((B, H), np.float32)
    for c, r in enumerate(res.results):
        o = r["o"].reshape(EPC, 128, HT, n_pad)  # [e, p, t, j]
        for i in range(EPC):
            e = c * EPC + i
            tk = toks[e]
            contrib = o[i, :, :, : len(tk)].transpose(2, 1, 0).reshape(len(tk), H)
            y[tk] += contrib
    y *= np.asarray(x_active_mask).astype(np.float32)[:, None]
    if _trace:
        kernel.last_results = res
    return y


# revision 21
# speedup vs baseline: 1.8079x; 1.2197x over previous
"""Trainium2 Bass kernel for nn_DecodeMoeOps (MoE decode: dispatch-quant,
grouped int8 GEMM1, SwiGLU, requant, grouped int8 GEMM2, weighted combine).

Expert-parallel across 8 NeuronCores: core c owns experts {2c, 2c+1}. Each
core computes only the tokens routed to its experts (gathered host-side,
padded to N_PAD), using weight-stationary GEMMs over routed tokens:

  GEMM1: out[f, tok] = w1_tile[k,f].T @ xqs[k, tok]   (xqs = fp16(xq*sx))
  GEMM2: out[h, tok] = w2_tile[i,h].T @ aq[i, tok]

w1 ships as int8 and is cast to fp16 on-chip (split across DVE/ACT/GPSIMD);
w2 ships as fp8e3m4 (exact for |w|<=31, max abs err 2 above) with the 16x
scale folded into w2_scale. Per-channel dequant scales are per-partition in
this layout; the requant absmax runs on GPSIMD partition_all_reduce. Host
scatters the per-expert [h, tok] outputs back into y[B, H].
"""

import os
import sys

for _p in ("/opt/trn_rl_repo", "/root/.axon_site/_ro/trn_rl_repo"):
    if os.path.isdir(_p) and _p not in sys.path:
        sys.path.insert(0, _p)

from contextlib import ExitStack

import ml_dtypes
import numpy as np

import concourse.bass as bass
import concourse.mybir as mybir
import concourse.tile as tile
from concourse import bacc
from concourse import bass_isa
from concourse.bass_utils import run_bass_kernel_spmd

B, TOPK, H, I, E = 128, 8, 2048, 1408, 16
NCORES = 8
EPC = E // NCORES  # experts per core
KH = H // 128  # 16 k-tiles for GEMM1 contraction
KI = I // 128  # 11 k-tiles for GEMM2 contraction
FT = I // 128  # 11 f-tiles per GEMM1 half
HT = H // 128  # 16 h-tiles for GEMM2 output
I2 = 2 * I
F32 = mybir.dt.float32
BF16 = mybir.dt.bfloat16
F16 = mybir.dt.float16
I8 = mybir.dt.int8
F8E3 = mybir.dt.float8e3
MAGIC = float(3 * 2**22)  # fp32 round-to-int magic (covers negatives)

# on-chip int8->fp16 cast: free-dim split of each [128, 2816] w1 k-tile
CAST_DVE = (0, 1536)
CAST_ACT = (1536, 704)
CAST_POOL = (2240, 576)

_cache: dict = {}


def _build_program(n_pad: int):
    mult = mybir.AluOpType.mult
    nc = bacc.Bacc(
        "TRN2",
        target_bir_lowering=False,
        debug=False,
        num_devices=NCORES,
    )

    FW = FT * n_pad   # gate/up accumulator width
    HW = HT * n_pad   # GEMM2 accumulator width

    # --- per-core DRAM I/O ---
    xqsT_d = nc.dram_tensor("xqsT", [EPC, 128, KH, n_pad], F16, kind="ExternalInput").ap()
    w1_d = nc.dram_tensor("w1t", [EPC, KH, 128, I2], I8, kind="ExternalInput").ap()
    w2_d = nc.dram_tensor("w2t", [EPC, KI, 128, H], F8E3, kind="ExternalInput").ap()
    s1g_d = nc.dram_tensor("s1g", [EPC, 128, FT], F32, kind="ExternalInput").ap()
    s1u_d = nc.dram_tensor("s1u", [EPC, 128, FT], F32, kind="ExternalInput").ap()
    w2s_d = nc.dram_tensor("scale2", [EPC, 128, HT], F32, kind="ExternalInput").ap()
    comb_d = nc.dram_tensor("combs", [EPC, 128, n_pad], F32, kind="ExternalInput").ap()
    o_d = nc.dram_tensor("o", [EPC, 128, HW], F32, kind="ExternalOutput").ap()

    with tile.TileContext(nc) as tc, ExitStack() as ctx:
        consts = ctx.enter_context(tc.tile_pool(name="consts", bufs=1))
        w1i8p = ctx.enter_context(tc.tile_pool(name="w1i8", bufs=5))
        w1f16p = ctx.enter_context(tc.tile_pool(name="w1f16", bufs=5))
        w2p = ctx.enter_context(tc.tile_pool(name="w2p", bufs=1))
        epi = ctx.enter_context(tc.tile_pool(name="epi", bufs=2))
        op_ = ctx.enter_context(tc.tile_pool(name="op", bufs=2))
        ps1_pool = ctx.enter_context(tc.tile_pool(name="ps1", bufs=2, space="PSUM"))
        ps2_pool = ctx.enter_context(tc.tile_pool(name="ps2", bufs=1, space="PSUM"))

        # --- prologue: small inputs ---
        xqs_s = consts.tile([128, EPC, KH, n_pad], F16, name="xqs_s")
        nc.scalar.dma_start(out=xqs_s[:], in_=xqsT_d.rearrange("e p k j -> p e k j"))
        s1g_s = consts.tile([128, EPC, FT], F32, name="s1g_s")
        nc.scalar.dma_start(out=s1g_s[:], in_=s1g_d.rearrange("e p t -> p e t"))
        s1u_s = consts.tile([128, EPC, FT], F32, name="s1u_s")
        nc.scalar.dma_start(out=s1u_s[:], in_=s1u_d.rearrange("e p t -> p e t"))
        w2s_s = consts.tile([128, EPC, HT], F32, name="w2s_s")
        nc.scalar.dma_start(out=w2s_s[:], in_=w2s_d.rearrange("e p t -> p e t"))
        comb_s = consts.tile([128, EPC, n_pad], F32, name="comb_s")
        nc.scalar.dma_start(out=comb_s[:], in_=comb_d.rearrange("e p j -> p e j"))

        def bank_flags(offsets_bytes):
            """PSUM accumulation start/stop flags per chunk: matmul start=True
            zeroes the whole 2KB bank, so exactly one start (first chunk) and
            one stop (last chunk) per bank. Offsets must not cross banks."""
            first, last = {}, {}
            for i, off in enumerate(offsets_bytes):
                b = off // 2048
                if b not in first:
                    first[b] = i
                last[b] = i
            starts = {i for i in first.values()}
            stops = {i for i in last.values()}
            return starts, stops

        def epilogue1_ops(e, ps1_e, out, last=False):
            """dequant + SwiGLU + requant -> aq; returns one closure per op
            so the caller can interleave emission with other work. For the
            final expert (last=True) the requant is chunked per GEMM2 k-tile
            and kept off GPSIMD so the tail chain is as short as possible."""
            ps_g = ps1_e[:, 0:FW]
            ps_u = ps1_e[:, FW : 2 * FW]
            s1g_b = epi.tile([128, FT, n_pad], F32, tag="s1gb", name=f"s1gb_{e}")
            s1u_b = epi.tile([128, FT, n_pad], F32, tag="s1ub", name=f"s1ub_{e}")
            gate = epi.tile([128, FW], F32, tag="gate", name=f"gate_{e}")
            sig = epi.tile([128, FW], F32, tag="sig", name=f"sig_{e}")
            gdu = epi.tile([128, FW], F32, tag="gdu", name=f"gdu_{e}")
            t1 = epi.tile([128, FW], F32, tag="t1", name=f"t1_{e}")
            act2 = epi.tile([128, FW], F32, tag="act2", name=f"act2_{e}")
            am = epi.tile([128, FW], F32, tag="am", name=f"am_{e}")
            m = epi.tile([128, n_pad], F32, tag="m", name=f"m_{e}")
            mc = epi.tile([128, n_pad], F32, tag="mc", name=f"mc_{e}")
            r = epi.tile([128, n_pad], F32, tag="r", name=f"r_{e}")
            tq = epi.tile([128, FW], F32, tag="tq", name=f"tq_{e}")
            tq2 = epi.tile([128, FW], F32, tag="tq2", name=f"tq2_{e}")
            aq = epi.tile([128, FT, n_pad], BF16, tag="aq", name=f"aq_{e}")
            s2c = epi.tile([128, n_pad], F32, tag="s2c", name=f"s2c_{e}")
            w2sc = epi.tile([128, HT, n_pad], F32, tag="w2sc", name=f"w2sc_{e}")
            out["aq"], out["w2sc"] = aq, w2sc
            ops = [
                lambda: nc.vector.tensor_copy(
                    out=s1g_b[:],
                    in_=s1g_s[:, e, :].unsqueeze(2).broadcast_to([128, FT, n_pad])),
                lambda: nc.vector.tensor_copy(
                    out=s1u_b[:],
                    in_=s1u_s[:, e, :].unsqueeze(2).broadcast_to([128, FT, n_pad])),
                lambda: nc.vector.tensor_tensor(
                    out=gate[:], in0=ps_g,
                    in1=s1g_b[:].rearrange("p t n -> p (t n)"), op=mult),
                lambda: nc.scalar.activation(
                    out=sig[:], in_=gate[:],
                    func=mybir.ActivationFunctionType.Sigmoid),
                lambda: nc.vector.tensor_tensor(
                    out=gdu[:], in0=gate[:],
                    in1=s1u_b[:].rearrange("p t n -> p (t n)"), op=mult),
                (lambda: nc.vector.tensor_tensor(
                    out=t1[:], in0=gdu[:], in1=sig[:], op=mult)) if last else
                (lambda: nc.gpsimd.tensor_tensor(
                    out=t1[:], in0=gdu[:], in1=sig[:], op=mult)),
                lambda: nc.vector.tensor_tensor(
                    out=act2[:], in0=t1[:], in1=ps_u, op=mult),
                lambda: nc.gpsimd.partition_all_reduce(
                    am[:], act2[:], channels=128,
                    reduce_op=bass_isa.ReduceOp.absmax),
                lambda: nc.vector.tensor_reduce(
                    out=m[:], in_=am[:].rearrange("p (t n) -> p n t", t=FT),
                    op=mybir.AluOpType.max, axis=mybir.AxisListType.X),
                lambda: nc.vector.tensor_scalar_max(
                    out=mc[:], in0=m[:], scalar1=1e-12),
                lambda: nc.vector.reciprocal(out=r[:], in_=mc[:]),
            ]
            tqv = tq[:].rearrange("p (t n) -> p t n", t=FT)
            tq2v = tq2[:].rearrange("p (t n) -> p t n", t=FT)
            a2v = act2[:].rearrange("p (t n) -> p t n", t=FT)
            if not last:
                ops += [
                    lambda: nc.vector.scalar_tensor_tensor(
                        out=tqv, in0=a2v, scalar=127.0,
                        in1=r[:].unsqueeze(1).broadcast_to([128, FT, n_pad]),
                        op0=mult, op1=mult),
                    lambda: nc.scalar.activation(
                        out=tq2[:], in_=tq[:],
                        func=mybir.ActivationFunctionType.Copy, bias=MAGIC),
                    lambda: nc.scalar.activation(
                        out=aq[:].rearrange("p t n -> p (t n)"), in_=tq2[:],
                        func=mybir.ActivationFunctionType.Copy, bias=-MAGIC),
                ]
            else:
                # per-GEMM2-k-tile requant: aq chunk ki is ready ~3 small ops
                # after r, so GEMM2 can start immediately
                for ki in range(KI):
                    ops += [
                        (lambda ki=ki: nc.vector.scalar_tensor_tensor(
                            out=tqv[:, ki : ki + 1, :],
                            in0=a2v[:, ki : ki + 1, :], scalar=127.0,
                            in1=r[:].unsqueeze(1).broadcast_to([128, 1, n_pad]),
                            op0=mult, op1=mult)),
                        (lambda ki=ki: nc.scalar.activation(
                            out=tq2v[:, ki : ki + 1, :], in_=tqv[:, ki : ki + 1, :],
                            func=mybir.ActivationFunctionType.Copy, bias=MAGIC)),
                        (lambda ki=ki: nc.vector.tensor_scalar_add(
                            out=aq[:, ki : ki + 1, :], in0=tq2v[:, ki : ki + 1, :],
                            scalar1=-MAGIC)),
                    ]
            ops += [
                lambda: nc.vector.scalar_tensor_tensor(
                    out=s2c[:], in0=mc[:], scalar=1.0 / 127.0,
                    in1=comb_s[:, e, :], op0=mult, op1=mult),
                lambda: nc.gpsimd.tensor_tensor(
                    out=w2sc[:],
                    in0=w2s_s[:, e, :].unsqueeze(2).broadcast_to([128, HT, n_pad]),
                    in1=s2c[:].unsqueeze(1).broadcast_to([128, HT, n_pad]),
                    op=mult),
            ]
            return ops

        def gemm2(e, aq, w2sc, w2tiles):
            """weight-stationary GEMM2 + per-bank dequant + output DMA.
            PSUM can only be read once a bank's accumulation group stopped,
            so the last-ki MMs and the dequant proceed bank by bank."""
            ps2 = ps2_pool.tile([128, HW], F32, tag="ps2", name=f"ps2_{e}")
            o_sb = op_.tile([128, HW], F32, tag="o_sb", name=f"o_{e}")
            cpb = max(1, 2048 // (n_pad * 4))  # h-chunks per PSUM bank
            banks = [list(range(b, min(b + cpb, HT))) for b in range(0, HT, cpb)]
            for ki in range(KI):
                last = ki == KI - 1
                for bi, bchunks in enumerate(banks):
                    for t in bchunks:
                        nc.tensor.matmul(
                            ps2[:, t * n_pad : (t + 1) * n_pad],
                            lhsT=w2tiles[ki][:, t * 128 : (t + 1) * 128],
                            rhs=aq[:, ki, :],
                            start=(ki == 0 and t in g2_starts),
                            stop=(last and t in g2_stops),
                        )
                    if not last:
                        continue
                    lo = bchunks[0] * n_pad
                    hi = (bchunks[-1] + 1) * n_pad
                    if bi < len(banks) - 1:
                        # earlier banks: ACT copies psum out, GPSIMD scales
                        od = epi.tile([128, hi - lo], F32, tag="odeq", name=f"od_{e}_{bi}")
                        nc.scalar.activation(
                            out=od[:], in_=ps2[:, lo:hi],
                            func=mybir.ActivationFunctionType.Copy,
                        )
                        nc.gpsimd.tensor_tensor(
                            out=o_sb[:, lo:hi],
                            in0=od[:],
                            in1=w2sc[:, bchunks[0] : bchunks[-1] + 1, :].rearrange(
                                "p t n -> p (t n)"
                            ),
                            op=mult,
                        )
                    else:
                        # final bank: one DVE op for the shortest tail chain
                        nc.vector.tensor_tensor(
                            out=o_sb[:, lo:hi],
                            in0=ps2[:, lo:hi],
                            in1=w2sc[:, bchunks[0] : bchunks[-1] + 1, :].rearrange(
                                "p t n -> p (t n)"
                            ),
                            op=mult,
                        )
                    nc.scalar.dma_start(
                        out=o_d[e, :, lo:hi], in_=o_sb[:, lo:hi]
                    )

        # --- main pipeline: expert-sequential so epilogue(e0) hides under
        # --- expert 1's cast stream; w2(e1) is the last DMA (tail-paced)
        g1_chunks = [(h, t) for h in (0, 1) for t in range(FT)]
        g1_chunks_last = [(h, t) for h in (1, 0) for t in range(FT)]
        s_idx, _ = bank_flags([h * FW * 4 + t * n_pad * 4 for h, t in g1_chunks])
        _, e_idx = bank_flags([h * FW * 4 + t * n_pad * 4 for h, t in g1_chunks_last])
        g1_starts = {g1_chunks[i] for i in s_idx}
        g1_stops = {g1_chunks_last[i] for i in e_idx}
        g2_starts, g2_stops = bank_flags([t * n_pad * 4 for t in range(HT)])
        w2tiles = {}
        epi_res = {e: {} for e in range(EPC)}
        pending = []
        for e in range(EPC):
            ps1_e = ps1_pool.tile([128, 2 * FW], F32, tag="ps1", name=f"ps1_{e}")
            for k in range(KH):
                w1i8 = w1i8p.tile([128, I2], I8, tag="w1i8", name=f"w1i8_{e}_{k}")
                nc.sync.dma_start(out=w1i8[:], in_=w1_d[e, k])
                w1f = w1f16p.tile([128, I2], F16, tag="w1f", name=f"w1f_{e}_{k}")
                o0, n0 = CAST_DVE
                nc.vector.tensor_copy(out=w1f[:, o0 : o0 + n0], in_=w1i8[:, o0 : o0 + n0])
                o1, n1 = CAST_ACT
                nc.scalar.activation(
                    out=w1f[:, o1 : o1 + n1],
                    in_=w1i8[:, o1 : o1 + n1],
                    func=mybir.ActivationFunctionType.Copy,
                )
                o2, n2 = CAST_POOL
                nc.gpsimd.tensor_copy(out=w1f[:, o2 : o2 + n2], in_=w1i8[:, o2 : o2 + n2])
                rhs = xqs_s[:, e, k, :]
                chunks = g1_chunks if k < KH - 1 else g1_chunks_last
                for half, t in chunks:
                    base = half * FW
                    nc.tensor.matmul(
                        ps1_e[:, base + t * n_pad : base + (t + 1) * n_pad],
                        lhsT=w1f[:, half * I + t * 128 : half * I + (t + 1) * 128],
                        rhs=rhs,
                        start=(k == 0 and (half, t) in g1_starts),
                        stop=(k == KH - 1 and (half, t) in g1_stops),
                    )
                # sprinkle the previous expert's epilogue into this cast
                # stream so its cross-engine chain never stalls the casts
                if pending and k >= 1:
                    pending.pop(0)()
            while pending:
                pending.pop(0)()
            pending = epilogue1_ops(e, ps1_e, epi_res[e], last=(e == EPC - 1))
        # all w2 DMAs ride behind the full w1 stream in the SP queue
        for e in range(EPC):
            w2tiles[e] = []
            for ki in range(KI):
                w2t = w2p.tile([128, H], F8E3, tag=f"w2_{e}_{ki}", name=f"w2_{e}_{ki}")
                nc.sync.dma_start(out=w2t[:], in_=w2_d[e, ki])
                w2tiles[e].append(w2t)
        # last expert's epilogue emitted inline
        while pending:
            pending.pop(0)()
        for e in range(EPC):
            gemm2(e, epi_res[e]["aq"], epi_res[e]["w2sc"], w2tiles[e])

    nc.compile()
    return nc


def get_program(n_pad: int):
    key = ("nc", n_pad)
    if key not in _cache:
        _cache[key] = _build_program(n_pad)
    return _cache[key]


def _routing(expert_ids, expert_scales):
    """comb[B, E] scatter-add; token lists per expert; N_PAD."""
    comb = np.zeros((B, E), np.float32)
    np.add.at(comb, (np.arange(B)[:, None], np.asarray(expert_ids)),
              np.asarray(expert_scales, np.float32))
    routed = np.zeros((B, E), bool)
    routed[np.arange(B)[:, None], np.asarray(expert_ids)] = True
    toks = [np.nonzero(routed[:, e])[0] for e in range(E)]
    max_n = max(len(t) for t in toks)
    n_pad = 16
    while n_pad < max_n:
        n_pad *= 2
    # PSUM chunking requires pow2 n_pad; >64 would overflow the 8 banks
    assert n_pad <= 64, f"routing too dense for this kernel: n_pad={n_pad}"
    return comb, toks, n_pad


def _prep_inputs(x, expert_ids, smooth_scales, expert_scales, w1, w1_scale, w2, w2_scale):
    """Host-side dispatch: quantize x, route tokens, shard experts."""
    x = np.asarray(x, np.float32)
    smooth_scales = np.asarray(smooth_scales, np.float32)
    w1_scale = np.asarray(w1_scale, np.float32)
    w2_scale = np.asarray(w2_scale, np.float32)

    # dynamic per-token int8 quantization (exact mirror of reference ops)
    sx = np.maximum(np.max(np.abs(x), axis=-1, keepdims=True), 1e-12) / 127.0
    xq = np.round(np.clip(x / sx, -128.0, 127.0)).astype(np.float32)
    xqs = (xq * sx).astype(np.float16)  # [B, H]
    xqsT = np.ascontiguousarray(
        xqs.T.reshape(KH, 128, B).transpose(1, 0, 2)
    )  # [128, KH, B]

    comb, toks, n_pad = _routing(expert_ids, expert_scales)

    w1v = np.asarray(w1).astype(np.int8)
    w2v = np.asarray(w2).astype(np.int8)

    in_maps = []
    for c in range(NCORES):
        es = list(range(c * EPC, (c + 1) * EPC))
        xqsT_e = np.zeros((EPC, 128, KH, n_pad), np.float16)
        comb_e = np.zeros((EPC, 128, n_pad), np.float32)
        for i, e in enumerate(es):
            tk = toks[e]
            xqsT_e[i, :, :, : len(tk)] = xqsT[:, :, tk]
            comb_e[i, :, : len(tk)] = comb[tk, e][None, :]
        w1c = w1v[es].reshape(EPC, KH, 128, I2)
        w2c = np.ascontiguousarray(
            (w2v[es].reshape(EPC, KI, 128, H).astype(np.float32) / 16.0)
        ).astype(ml_dtypes.float8_e3m4)
        # per-partition scale columns [e, p, T]
        s1g_full = w1_scale[es][:, :I]
        s1u_full = w1_scale[es][:, I:] * smooth_scales[es]
        s1g = np.ascontiguousarray(s1g_full.reshape(EPC, FT, 128).transpose(0, 2, 1))
        s1u = np.ascontiguousarray(s1u_full.reshape(EPC, FT, 128).transpose(0, 2, 1))
        sc2 = np.ascontiguousarray(
            (w2_scale[es] * 16.0).reshape(EPC, HT, 128).transpose(0, 2, 1))
        in_maps.append(
            {
                "xqsT": xqsT_e,
                "w1t": np.ascontiguousarray(w1c),
                "w2t": w2c,
                "s1g": s1g.astype(np.float32),
                "s1u": s1u.astype(np.float32),
                "scale2": sc2.astype(np.float32),
                "combs": comb_e,
            }
        )
    return in_maps, toks, n_pad


def kernel(
    x,
    expert_ids,
    smooth_scales,
    expert_scales,
    x_active_mask,
    w1,
    w1_scale,
    w2,
    w2_scale,
    _trace=False,
    _trace_kwargs=None,
):
    in_maps, toks, n_pad = _prep_inputs(
        x, expert_ids, smooth_scales, expert_scales, w1, w1_scale, w2, w2_scale
    )
    nc = get_program(n_pad)
    res = run_bass_kernel_spmd(
        nc,
        in_maps,
        core_ids=list(range(NCORES)),
        trace=_trace,
        **(_trace_kwargs or {}),
    )
    y = np.zeros((B, H), np.float32)
    for c, r in enumerate(res.results):
        o = r["o"].reshape(EPC, 128, HT, n_pad)  # [e, p, t, j]
        for i in range(EPC):
            e = c * EPC + i
            tk = toks[e]
            contrib = o[i, :, :, : len(tk)].transpose(2, 1, 0).reshape(len(tk), H)
            y[tk] += contrib
    y *= np.asarray(x_active_mask).astype(np.float32)[:, None]
    if _trace:
        kernel.last_results = res
    return y


# revision 32
# speedup vs baseline: 1.8086x; 1.0004x over previous
"""Trainium2 Bass kernel for nn_DecodeMoeOps (MoE decode: dispatch-quant,
grouped int8 GEMM1, SwiGLU, requant, grouped int8 GEMM2, weighted combine).

Expert-parallel across 8 NeuronCores: core c owns experts {2c, 2c+1}. Each
core computes only the tokens routed to its experts (gathered host-side,
padded to N_PAD), using weight-stationary GEMMs over routed tokens:

  GEMM1: out[f, tok] = w1_tile[k,f].T @ xqs[k, tok]   (xqs = fp16(xq*sx))
  GEMM2: out[h, tok] = w2_tile[i,h].T @ aq[i, tok]

w1 ships as int8 and is cast to fp16 on-chip (split across DVE/ACT/GPSIMD);
w2 ships as fp8e3m4 (exact for |w|<=31, max abs err 2 above) with the 16x
scale folded into w2_scale. Per-channel dequant scales are per-partition in
this layout; the requant absmax runs on GPSIMD partition_all_reduce. Host
scatters the per-expert [h, tok] outputs back into y[B, H].
"""

import os
import sys

for _p in ("/opt/trn_rl_repo", "/root/.axon_site/_ro/trn_rl_repo"):
    if os.path.isdir(_p) and _p not in sys.path:
        sys.path.insert(0, _p)

from contextlib import ExitStack

import ml_dtypes
import numpy as np

import concourse.bass as bass
import concourse.mybir as mybir
import concourse.tile as tile
from concourse import bacc
from concourse import bass_isa
from concourse.bass_utils import run_bass_kernel_spmd

B, TOPK, H, I, E = 128, 8, 2048, 1408, 16
NCORES = 8
EPC = E // NCORES  # experts per core
KH = H // 128  # 16 k-tiles for GEMM1 contraction
KI = I // 128  # 11 k-tiles for GEMM2 contraction
FT = I // 128  # 11 f-tiles per GEMM1 half
HT = H // 128  # 16 h-tiles for GEMM2 output
I2 = 2 * I
F32 = mybir.dt.float32
BF16 = mybir.dt.bfloat16
F16 = mybir.dt.float16
I8 = mybir.dt.int8
F8E3 = mybir.dt.float8e3
MAGIC = float(3 * 2**22)  # fp32 round-to-int magic (covers negatives)

# on-chip int8->fp16 cast: free-dim split of each [128, 2816] w1 k-tile
CAST_DVE = (0, 1536)
CAST_ACT = (1536, 704)
CAST_POOL = (2240, 576)

_cache: dict = {}


def _build_program(n_pad: int):
    mult = mybir.AluOpType.mult
    nc = bacc.Bacc(
        "TRN2",
        target_bir_lowering=False,
        debug=False,
        num_devices=NCORES,
    )

    FW = FT * n_pad   # gate/up accumulator width
    HW = HT * n_pad   # GEMM2 accumulator width

    # --- per-core DRAM I/O ---
    xqsT_d = nc.dram_tensor("xqsT", [EPC, 128, KH, n_pad], F16, kind="ExternalInput").ap()
    w1_d = nc.dram_tensor("w1t", [EPC, KH, 128, I2], I8, kind="ExternalInput").ap()
    w2_d = nc.dram_tensor("w2t", [EPC, KI, 128, H], F8E3, kind="ExternalInput").ap()
    s1g_d = nc.dram_tensor("s1g", [EPC, 128, FT], F32, kind="ExternalInput").ap()
    s1gu_d = nc.dram_tensor("s1gu", [EPC, 128, FT], F32, kind="ExternalInput").ap()
    w2s_d = nc.dram_tensor("scale2", [EPC, 128, HT], F32, kind="ExternalInput").ap()
    comb_d = nc.dram_tensor("combs", [EPC, 128, n_pad], F32, kind="ExternalInput").ap()
    o_d = nc.dram_tensor("o", [EPC, 128, HW], BF16, kind="ExternalOutput").ap()

    with tile.TileContext(nc) as tc, ExitStack() as ctx:
        consts = ctx.enter_context(tc.tile_pool(name="consts", bufs=1))
        w1i8p = ctx.enter_context(tc.tile_pool(name="w1i8", bufs=5))
        w1f16p = ctx.enter_context(tc.tile_pool(name="w1f16", bufs=5))
        w2p = ctx.enter_context(tc.tile_pool(name="w2p", bufs=1))
        epi = ctx.enter_context(tc.tile_pool(name="epi", bufs=2))
        op_ = ctx.enter_context(tc.tile_pool(name="op", bufs=2))
        ps1_pool = ctx.enter_context(tc.tile_pool(name="ps1", bufs=2, space="PSUM"))
        ps2_pool = ctx.enter_context(tc.tile_pool(name="ps2", bufs=1, space="PSUM"))

        # --- prologue: small inputs ---
        xqs_s = consts.tile([128, EPC, KH, n_pad], F16, name="xqs_s")
        nc.scalar.dma_start(out=xqs_s[:], in_=xqsT_d.rearrange("e p k j -> p e k j"))
        s1g_s = consts.tile([128, EPC, FT], F32, name="s1g_s")
        nc.scalar.dma_start(out=s1g_s[:], in_=s1g_d.rearrange("e p t -> p e t"))
        s1gu_s = consts.tile([128, EPC, FT], F32, name="s1gu_s")
        nc.scalar.dma_start(out=s1gu_s[:], in_=s1gu_d.rearrange("e p t -> p e t"))
        w2s_s = consts.tile([128, EPC, HT], F32, name="w2s_s")
        nc.scalar.dma_start(out=w2s_s[:], in_=w2s_d.rearrange("e p t -> p e t"))
        comb_s = consts.tile([128, EPC, n_pad], F32, name="comb_s")
        nc.scalar.dma_start(out=comb_s[:], in_=comb_d.rearrange("e p j -> p e j"))

        def bank_flags(offsets_bytes):
            """PSUM accumulation start/stop flags per chunk: matmul start=True
            zeroes the whole 2KB bank, so exactly one start (first chunk) and
            one stop (last chunk) per bank. Offsets must not cross banks."""
            first, last = {}, {}
            for i, off in enumerate(offsets_bytes):
                b = off // 2048
                if b not in first:
                    first[b] = i
                last[b] = i
            starts = {i for i in first.values()}
            stops = {i for i in last.values()}
            return starts, stops

        def epilogue1_ops(e, ps1_e, out, last=False):
            """dequant + SwiGLU + requant -> aq; returns one closure per op
            so the caller can interleave emission with other work. For the
            final expert (last=True) the requant is chunked per GEMM2 k-tile
            and kept off GPSIMD so the tail chain is as short as possible."""
            ps_g = ps1_e[:, 0:FW]
            ps_u = ps1_e[:, FW : 2 * FW]
            s1g_b = epi.tile([128, FT, n_pad], F32, tag="s1gb", name=f"s1gb_{e}")
            s1gu_b = epi.tile([128, FT, n_pad], F32, tag="s1gub", name=f"s1gub_{e}")
            gate = epi.tile([128, FW], F32, tag="gate", name=f"gate_{e}")
            sig = epi.tile([128, FW], F32, tag="sig", name=f"sig_{e}")
            gdu = epi.tile([128, FW], F32, tag="gdu", name=f"gdu_{e}")
            t1 = epi.tile([128, FW], F32, tag="t1", name=f"t1_{e}")
            act2 = epi.tile([128, FW], F32, tag="act2", name=f"act2_{e}")
            am = epi.tile([128, FW], F32, tag="am", name=f"am_{e}")
            m = epi.tile([128, n_pad], F32, tag="m", name=f"m_{e}")
            mc = epi.tile([128, n_pad], F32, tag="mc", name=f"mc_{e}")
            r = epi.tile([128, n_pad], F32, tag="r", name=f"r_{e}")
            tq = epi.tile([128, FW], F32, tag="tq", name=f"tq_{e}")
            tq2 = epi.tile([128, FW], F32, tag="tq2", name=f"tq2_{e}")
            aq = epi.tile([128, FT, n_pad], BF16, tag="aq", name=f"aq_{e}")
            s2c = epi.tile([128, n_pad], F32, tag="s2c", name=f"s2c_{e}")
            w2sc = epi.tile([128, HT, n_pad], F32, tag="w2sc", name=f"w2sc_{e}")
            out["aq"], out["w2sc"] = aq, w2sc
            ops = [
                lambda: nc.vector.tensor_copy(
                    out=s1g_b[:],
                    in_=s1g_s[:, e, :].unsqueeze(2).broadcast_to([128, FT, n_pad])),
                lambda: nc.vector.tensor_copy(
                    out=s1gu_b[:],
                    in_=s1gu_s[:, e, :].unsqueeze(2).broadcast_to([128, FT, n_pad])),
                lambda: nc.vector.tensor_tensor(
                    out=gate[:], in0=ps_g,
                    in1=s1g_b[:].rearrange("p t n -> p (t n)"), op=mult),
                lambda: nc.scalar.activation(
                    out=sig[:], in_=gate[:],
                    func=mybir.ActivationFunctionType.Sigmoid),
                lambda: nc.vector.tensor_tensor(
                    out=gdu[:], in0=ps_g,
                    in1=s1gu_b[:].rearrange("p t n -> p (t n)"), op=mult),
                (lambda: nc.vector.tensor_tensor(
                    out=t1[:], in0=gdu[:], in1=sig[:], op=mult)) if last else
                (lambda: nc.gpsimd.tensor_tensor(
                    out=t1[:], in0=gdu[:], in1=sig[:], op=mult)),
                lambda: nc.vector.tensor_tensor(
                    out=act2[:], in0=t1[:], in1=ps_u, op=mult),
                lambda: nc.gpsimd.partition_all_reduce(
                    am[:], act2[:], channels=128,
                    reduce_op=bass_isa.ReduceOp.absmax),
                lambda: nc.vector.tensor_reduce(
                    out=m[:], in_=am[:].rearrange("p (t n) -> p n t", t=FT),
                    op=mybir.AluOpType.max, axis=mybir.AxisListType.X),
                lambda: nc.vector.tensor_scalar_max(
                    out=mc[:], in0=m[:], scalar1=1e-12),
                lambda: nc.vector.reciprocal(out=r[:], in_=mc[:]),
            ]
            tqv = tq[:].rearrange("p (t n) -> p t n", t=FT)
            tq2v = tq2[:].rearrange("p (t n) -> p t n", t=FT)
            a2v = act2[:].rearrange("p (t n) -> p t n", t=FT)
            if not last:
                ops += [
                    lambda: nc.vector.scalar_tensor_tensor(
                        out=tqv, in0=a2v, scalar=127.0,
                        in1=r[:].unsqueeze(1).broadcast_to([128, FT, n_pad]),
                        op0=mult, op1=mult),
                    lambda: nc.scalar.activation(
                        out=tq2[:], in_=tq[:],
                        func=mybir.ActivationFunctionType.Copy, bias=MAGIC),
                    lambda: nc.scalar.activation(
                        out=aq[:].rearrange("p t n -> p (t n)"), in_=tq2[:],
                        func=mybir.ActivationFunctionType.Copy, bias=-MAGIC),
                ]
            else:
                # per-GEMM2-k-tile requant: aq chunk ki is ready ~3 small ops
                # after r, so GEMM2 can start immediately
                for ki in range(KI):
                    ops += [
                        (lambda ki=ki: nc.vector.scalar_tensor_tensor(
                            out=tqv[:, ki : ki + 1, :],
                            in0=a2v[:, ki : ki + 1, :], scalar=127.0,
                            in1=r[:].unsqueeze(1).broadcast_to([128, 1, n_pad]),
                            op0=mult, op1=mult)),
                        (lambda ki=ki: nc.scalar.activation(
                            out=tq2v[:, ki : ki + 1, :], in_=tqv[:, ki : ki + 1, :],
                            func=mybir.ActivationFunctionType.Copy, bias=MAGIC)),
                        (lambda ki=ki: nc.vector.tensor_scalar_add(
                            out=aq[:, ki : ki + 1, :], in0=tq2v[:, ki : ki + 1, :],
                            scalar1=-MAGIC)),
                    ]
            ops += [
                lambda: nc.vector.scalar_tensor_tensor(
                    out=s2c[:], in0=mc[:], scalar=1.0 / 127.0,
                    in1=comb_s[:, e, :], op0=mult, op1=mult),
                lambda: nc.gpsimd.tensor_tensor(
                    out=w2sc[:],
                    in0=w2s_s[:, e, :].unsqueeze(2).broadcast_to([128, HT, n_pad]),
                    in1=s2c[:].unsqueeze(1).broadcast_to([128, HT, n_pad]),
                    op=mult),
            ]
            return ops

        def gemm2(e, aq, w2sc, w2tiles):
            """weight-stationary GEMM2 + per-bank dequant + output DMA.
            PSUM can only be read once a bank's accumulation group stopped,
            so the last-ki MMs and the dequant proceed bank by bank."""
            ps2 = ps2_pool.tile([128, HW], F32, tag="ps2", name=f"ps2_{e}")
            o_sb = op_.tile([128, HW], BF16, tag="o_sb", name=f"o_{e}")
            cpb = max(1, 2048 // (n_pad * 4))  # h-chunks per PSUM bank
            banks = [list(range(b, min(b + cpb, HT))) for b in range(0, HT, cpb)]
            for ki in range(KI):
                last = ki == KI - 1
                for bi, bchunks in enumerate(banks):
                    for t in bchunks:
                        nc.tensor.matmul(
                            ps2[:, t * n_pad : (t + 1) * n_pad],
                            lhsT=w2tiles[ki][:, t * 128 : (t + 1) * 128],
                            rhs=aq[:, ki, :],
                            start=(ki == 0 and t in g2_starts),
                            stop=(last and t in g2_stops),
                        )
                    if not last:
                        continue
                    lo = bchunks[0] * n_pad
                    hi = (bchunks[-1] + 1) * n_pad
                    if bi < len(banks) - 1:
                        # earlier banks: ACT copies psum out, GPSIMD scales
                        od = epi.tile([128, hi - lo], F32, tag="odeq", name=f"od_{e}_{bi}")
                        nc.scalar.activation(
                            out=od[:], in_=ps2[:, lo:hi],
                            func=mybir.ActivationFunctionType.Copy,
                        )
                        nc.gpsimd.tensor_tensor(
                            out=o_sb[:, lo:hi],
                            in0=od[:],
                            in1=w2sc[:, bchunks[0] : bchunks[-1] + 1, :].rearrange(
                                "p t n -> p (t n)"
                            ),
                            op=mult,
                        )
                    else:
                        # final bank: one DVE op for the shortest tail chain
                        nc.vector.tensor_tensor(
                            out=o_sb[:, lo:hi],
                            in0=ps2[:, lo:hi],
                            in1=w2sc[:, bchunks[0] : bchunks[-1] + 1, :].rearrange(
                                "p t n -> p (t n)"
                            ),
                            op=mult,
                        )
                    nc.scalar.dma_start(
                        out=o_d[e, :, lo:hi], in_=o_sb[:, lo:hi]
                    )

        # --- main pipeline: expert-sequential so epilogue(e0) hides under
        # --- expert 1's cast stream; w2(e1) is the last DMA (tail-paced)
        g1_chunks = [(h, t) for h in (0, 1) for t in range(FT)]
        g1_chunks_last = [(h, t) for h in (1, 0) for t in range(FT)]
        s_idx, _ = bank_flags([h * FW * 4 + t * n_pad * 4 for h, t in g1_chunks])
        _, e_idx = bank_flags([h * FW * 4 + t * n_pad * 4 for h, t in g1_chunks_last])
        g1_starts = {g1_chunks[i] for i in s_idx}
        g1_stops = {g1_chunks_last[i] for i in e_idx}
        g2_starts, g2_stops = bank_flags([t * n_pad * 4 for t in range(HT)])
        w2tiles = {}
        epi_res = {e: {} for e in range(EPC)}
        pending = []
        for e in range(EPC):
            ps1_e = ps1_pool.tile([128, 2 * FW], F32, tag="ps1", name=f"ps1_{e}")
            for k in range(KH):
                w1i8 = w1i8p.tile([128, I2], I8, tag="w1i8", name=f"w1i8_{e}_{k}")
                nc.sync.dma_start(out=w1i8[:], in_=w1_d[e, k])
                w1f = w1f16p.tile([128, I2], F16, tag="w1f", name=f"w1f_{e}_{k}")
                o0, n0 = CAST_DVE
                nc.vector.tensor_copy(out=w1f[:, o0 : o0 + n0], in_=w1i8[:, o0 : o0 + n0])
                o1, n1 = CAST_ACT
                nc.scalar.activation(
                    out=w1f[:, o1 : o1 + n1],
                    in_=w1i8[:, o1 : o1 + n1],
                    func=mybir.ActivationFunctionType.Copy,
                )
                o2, n2 = CAST_POOL
                nc.gpsimd.tensor_copy(out=w1f[:, o2 : o2 + n2], in_=w1i8[:, o2 : o2 + n2])
                rhs = xqs_s[:, e, k, :]
                chunks = g1_chunks if k < KH - 1 else g1_chunks_last
                for half, t in chunks:
                    base = half * FW
                    nc.tensor.matmul(
                        ps1_e[:, base + t * n_pad : base + (t + 1) * n_pad],
                        lhsT=w1f[:, half * I + t * 128 : half * I + (t + 1) * 128],
                        rhs=rhs,
                        start=(k == 0 and (half, t) in g1_starts),
                        stop=(k == KH - 1 and (half, t) in g1_stops),
                    )
                # sprinkle the previous expert's epilogue into this cast
                # stream so its cross-engine chain never stalls the casts
                if pending and k >= 1:
                    pending.pop(0)()
            while pending:
                pending.pop(0)()
            pending = epilogue1_ops(e, ps1_e, epi_res[e], last=(e == EPC - 1))
        # all w2 DMAs ride behind the full w1 stream in the SP queue
        for e in range(EPC):
            w2tiles[e] = []
            for ki in range(KI):
                w2t = w2p.tile([128, H], F8E3, tag=f"w2_{e}_{ki}", name=f"w2_{e}_{ki}")
                nc.sync.dma_start(out=w2t[:], in_=w2_d[e, ki])
                w2tiles[e].append(w2t)
        # last expert's epilogue emitted inline
        while pending:
            pending.pop(0)()
        for e in range(EPC):
            gemm2(e, epi_res[e]["aq"], epi_res[e]["w2sc"], w2tiles[e])

    nc.compile()
    return nc


def get_program(n_pad: int):
    key = ("nc", n_pad)
    if key not in _cache:
        _cache[key] = _build_program(n_pad)
    return _cache[key]


def _routing(expert_ids, expert_scales):
    """comb[B, E] scatter-add; token lists per expert; N_PAD."""
    comb = np.zeros((B, E), np.float32)
    np.add.at(comb, (np.arange(B)[:, None], np.asarray(expert_ids)),
              np.asarray(expert_scales, np.float32))
    routed = np.zeros((B, E), bool)
    routed[np.arange(B)[:, None], np.asarray(expert_ids)] = True
    toks = [np.nonzero(routed[:, e])[0] for e in range(E)]
    max_n = max(len(t) for t in toks)
    n_pad = 16
    while n_pad < max_n:
        n_pad *= 2
    # PSUM chunking requires pow2 n_pad; >64 would overflow the 8 banks
    assert n_pad <= 64, f"routing too dense for this kernel: n_pad={n_pad}"
    return comb, toks, n_pad


def _prep_inputs(x, expert_ids, smooth_scales, expert_scales, w1, w1_scale, w2, w2_scale):
    """Host-side dispatch: quantize x, route tokens, shard experts."""
    x = np.asarray(x, np.float32)
    smooth_scales = np.asarray(smooth_scales, np.float32)
    w1_scale = np.asarray(w1_scale, np.float32)
    w2_scale = np.asarray(w2_scale, np.float32)

    # dynamic per-token int8 quantization (exact mirror of reference ops)
    sx = np.maximum(np.max(np.abs(x), axis=-1, keepdims=True), 1e-12) / 127.0
    xq = np.round(np.clip(x / sx, -128.0, 127.0)).astype(np.float32)
    xqs = (xq * sx).astype(np.float16)  # [B, H]
    xqsT = np.ascontiguousarray(
        xqs.T.reshape(KH, 128, B).transpose(1, 0, 2)
    )  # [128, KH, B]

    comb, toks, n_pad = _routing(expert_ids, expert_scales)

    w1v = np.asarray(w1).astype(np.int8)
    w2v = np.asarray(w2).astype(np.int8)

    in_maps = []
    for c in range(NCORES):
        es = list(range(c * EPC, (c + 1) * EPC))
        xqsT_e = np.zeros((EPC, 128, KH, n_pad), np.float16)
        comb_e = np.zeros((EPC, 128, n_pad), np.float32)
        for i, e in enumerate(es):
            tk = toks[e]
            xqsT_e[i, :, :, : len(tk)] = xqsT[:, :, tk]
            comb_e[i, :, : len(tk)] = comb[tk, e][None, :]
        w1c = w1v[es].reshape(EPC, KH, 128, I2)
        w2c = np.ascontiguousarray(
            (w2v[es].reshape(EPC, KI, 128, H).astype(np.float32) / 16.0)
        ).astype(ml_dtypes.float8_e3m4)
        # per-partition scale columns [e, p, T]
        s1g_full = w1_scale[es][:, :I]
        s1u_full = w1_scale[es][:, I:] * smooth_scales[es]
        s1g = np.ascontiguousarray(s1g_full.reshape(EPC, FT, 128).transpose(0, 2, 1))
        s1gu = np.ascontiguousarray(
            (s1g_full * s1u_full).reshape(EPC, FT, 128).transpose(0, 2, 1))
        sc2 = np.ascontiguousarray(
            (w2_scale[es] * 16.0).reshape(EPC, HT, 128).transpose(0, 2, 1))
        in_maps.append(
            {
                "xqsT": xqsT_e,
                "w1t": np.ascontiguousarray(w1c),
                "w2t": w2c,
                "s1g": s1g.astype(np.float32),
                "s1gu": s1gu.astype(np.float32),
                "scale2": sc2.astype(np.float32),
                "combs": comb_e,
            }
        )
    return in_maps, toks, n_pad


def kernel(
    x,
    expert_ids,
    smooth_scales,
    expert_scales,
    x_active_mask,
    w1,
    w1_scale,
    w2,
    w2_scale,
    _trace=False,
    _trace_kwargs=None,
):
    in_maps, toks, n_pad = _prep_inputs(
        x, expert_ids, smooth_scales, expert_scales, w1, w1_scale, w2, w2_scale
    )
    nc = get_program(n_pad)
    res = run_bass_kernel_spmd(
        nc,
        in_maps,
        core_ids=list(range(NCORES)),
        trace=_trace,
        **(_trace_kwargs or {}),
    )
    y = np.zeros((B, H), np.float32)
    for c, r in enumerate(res.results):
        o = np.asarray(r["o"], np.float32).reshape(EPC, 128, HT, n_pad)
        for i in range(EPC):
            e = c * EPC + i
            tk = toks[e]
            contrib = o[i, :, :, : len(tk)].transpose(2, 1, 0).reshape(len(tk), H)
            y[tk] += contrib
    y *= np.asarray(x_active_mask).astype(np.float32)[:, None]
    if _trace:
        kernel.last_results = res
    return y


# revision 36
# speedup vs baseline: 1.8284x; 1.0109x over previous
"""Trainium2 Bass kernel for nn_DecodeMoeOps (MoE decode: dispatch-quant,
grouped int8 GEMM1, SwiGLU, requant, grouped int8 GEMM2, weighted combine).

Expert-parallel across 8 NeuronCores: core c owns experts {2c, 2c+1}. Each
core computes only the tokens routed to its experts (gathered host-side,
padded to N_PAD), using weight-stationary GEMMs over routed tokens:

  GEMM1: out[f, tok] = w1_tile[k,f].T @ xqs[k, tok]   (xqs = fp16(xq*sx))
  GEMM2: out[h, tok] = w2_tile[i,h].T @ aq[i, tok]

w1 ships as int8 and is cast to fp16 on-chip (split across DVE/ACT/GPSIMD);
w2 ships as fp8e3m4 (exact for |w|<=31, max abs err 2 above) with the 16x
scale folded into w2_scale. Per-channel dequant scales are per-partition in
this layout; the requant absmax runs on GPSIMD partition_all_reduce. Host
scatters the per-expert [h, tok] outputs back into y[B, H].
"""

import os
import sys

for _p in ("/opt/trn_rl_repo", "/root/.axon_site/_ro/trn_rl_repo"):
    if os.path.isdir(_p) and _p not in sys.path:
        sys.path.insert(0, _p)

from contextlib import ExitStack

import ml_dtypes
import numpy as np

import concourse.bass as bass
import concourse.mybir as mybir
import concourse.tile as tile
from concourse import bacc
from concourse import bass_isa
from concourse.bass_utils import run_bass_kernel_spmd

B, TOPK, H, I, E = 128, 8, 2048, 1408, 16
NCORES = 8
EPC = E // NCORES  # experts per core
KH = H // 128  # 16 k-tiles for GEMM1 contraction
KI = I // 128  # 11 k-tiles for GEMM2 contraction
FT = I // 128  # 11 f-tiles per GEMM1 half
HT = H // 128  # 16 h-tiles for GEMM2 output
I2 = 2 * I
F32 = mybir.dt.float32
BF16 = mybir.dt.bfloat16
F16 = mybir.dt.float16
I8 = mybir.dt.int8
F8E3 = mybir.dt.float8e3
MAGIC = float(3 * 2**22)  # fp32 round-to-int magic (covers negatives)

# on-chip int8->fp16 cast: free-dim split of each [128, 2816] w1 k-tile
CAST_DVE = (0, 1536)
CAST_ACT = (1536, 704)
CAST_POOL = (2240, 576)

_cache: dict = {}


def _build_program(n_pad: int):
    mult = mybir.AluOpType.mult
    nc = bacc.Bacc(
        "TRN2",
        target_bir_lowering=False,
        debug=False,
        num_devices=NCORES,
    )

    FW = FT * n_pad   # gate/up accumulator width
    HW = HT * n_pad   # GEMM2 accumulator width

    # --- per-core DRAM I/O ---
    xqsT_d = nc.dram_tensor("xqsT", [EPC, 128, KH, n_pad], F16, kind="ExternalInput").ap()
    w1_d = nc.dram_tensor("w1t", [EPC, KH, 128, I2], I8, kind="ExternalInput").ap()
    w2_d = nc.dram_tensor("w2t", [EPC, KI, 128, H], F8E3, kind="ExternalInput").ap()
    s1g_d = nc.dram_tensor("s1g", [EPC, 128, FT], F32, kind="ExternalInput").ap()
    s1gu_d = nc.dram_tensor("s1gu", [EPC, 128, FT], F32, kind="ExternalInput").ap()
    w2s_d = nc.dram_tensor("scale2", [EPC, 128, HT], F32, kind="ExternalInput").ap()
    comb_d = nc.dram_tensor("combs", [EPC, 128, n_pad], F32, kind="ExternalInput").ap()
    o_d = nc.dram_tensor("o", [EPC, 128, HW], BF16, kind="ExternalOutput").ap()

    with tile.TileContext(nc) as tc, ExitStack() as ctx:
        consts = ctx.enter_context(tc.tile_pool(name="consts", bufs=1))
        w1i8p = ctx.enter_context(tc.tile_pool(name="w1i8", bufs=5))
        w1f16p = ctx.enter_context(tc.tile_pool(name="w1f16", bufs=5))
        w2p = ctx.enter_context(tc.tile_pool(name="w2p", bufs=1))
        epi = ctx.enter_context(tc.tile_pool(name="epi", bufs=2))
        op_ = ctx.enter_context(tc.tile_pool(name="op", bufs=2))
        ps1_pool = ctx.enter_context(tc.tile_pool(name="ps1", bufs=2, space="PSUM"))
        ps2_pool = ctx.enter_context(tc.tile_pool(name="ps2", bufs=1, space="PSUM"))

        # --- prologue: small inputs ---
        xqs_s = consts.tile([128, EPC, KH, n_pad], F16, name="xqs_s")
        nc.scalar.dma_start(out=xqs_s[:], in_=xqsT_d.rearrange("e p k j -> p e k j"))
        s1g_s = consts.tile([128, EPC, FT], F32, name="s1g_s")
        nc.scalar.dma_start(out=s1g_s[:], in_=s1g_d.rearrange("e p t -> p e t"))
        s1gu_s = consts.tile([128, EPC, FT], F32, name="s1gu_s")
        nc.scalar.dma_start(out=s1gu_s[:], in_=s1gu_d.rearrange("e p t -> p e t"))
        w2s_s = consts.tile([128, EPC, HT], F32, name="w2s_s")
        nc.scalar.dma_start(out=w2s_s[:], in_=w2s_d.rearrange("e p t -> p e t"))
        comb_s = consts.tile([128, EPC, n_pad], F32, name="comb_s")
        nc.scalar.dma_start(out=comb_s[:], in_=comb_d.rearrange("e p j -> p e j"))

        def bank_flags(offsets_bytes):
            """PSUM accumulation start/stop flags per chunk: matmul start=True
            zeroes the whole 2KB bank, so exactly one start (first chunk) and
            one stop (last chunk) per bank. Offsets must not cross banks."""
            first, last = {}, {}
            for i, off in enumerate(offsets_bytes):
                b = off // 2048
                if b not in first:
                    first[b] = i
                last[b] = i
            starts = {i for i in first.values()}
            stops = {i for i in last.values()}
            return starts, stops

        def epilogue1_ops(e, ps1_e, out, last=False):
            """dequant + SwiGLU + requant -> aq; returns one closure per op
            so the caller can interleave emission with other work. For the
            final expert (last=True) the requant is chunked per GEMM2 k-tile
            and kept off GPSIMD so the tail chain is as short as possible."""
            ps_g = ps1_e[:, 0:FW]
            ps_u = ps1_e[:, FW : 2 * FW]
            s1g_b = epi.tile([128, FT, n_pad], F32, tag="s1gb", name=f"s1gb_{e}")
            s1gu_b = epi.tile([128, FT, n_pad], F32, tag="s1gub", name=f"s1gub_{e}")
            gate = epi.tile([128, FW], F32, tag="gate", name=f"gate_{e}")
            sig = epi.tile([128, FW], F32, tag="sig", name=f"sig_{e}")
            gdu = epi.tile([128, FW], F32, tag="gdu", name=f"gdu_{e}")
            t1 = epi.tile([128, FW], F32, tag="t1", name=f"t1_{e}")
            act2 = epi.tile([128, FW], F32, tag="act2", name=f"act2_{e}")
            am = epi.tile([128, FW], F32, tag="am", name=f"am_{e}")
            m = epi.tile([128, n_pad], F32, tag="m", name=f"m_{e}")
            mc = epi.tile([128, n_pad], F32, tag="mc", name=f"mc_{e}")
            r = epi.tile([128, n_pad], F32, tag="r", name=f"r_{e}")
            tq = epi.tile([128, FW], F32, tag="tq", name=f"tq_{e}")
            tq2 = epi.tile([128, FW], F32, tag="tq2", name=f"tq2_{e}")
            aq = epi.tile([128, FT, n_pad], BF16, tag="aq", name=f"aq_{e}")
            s2c = epi.tile([128, n_pad], F32, tag="s2c", name=f"s2c_{e}")
            w2sc = epi.tile([128, HT, n_pad], F32, tag="w2sc", name=f"w2sc_{e}")
            out["aq"], out["w2sc"] = aq, w2sc
            ops = [
                lambda: nc.vector.tensor_copy(
                    out=s1g_b[:],
                    in_=s1g_s[:, e, :].unsqueeze(2).broadcast_to([128, FT, n_pad])),
                lambda: nc.vector.tensor_copy(
                    out=s1gu_b[:],
                    in_=s1gu_s[:, e, :].unsqueeze(2).broadcast_to([128, FT, n_pad])),
                lambda: nc.vector.tensor_tensor(
                    out=gate[:], in0=ps_g,
                    in1=s1g_b[:].rearrange("p t n -> p (t n)"), op=mult),
                lambda: nc.scalar.activation(
                    out=sig[:], in_=gate[:],
                    func=mybir.ActivationFunctionType.Sigmoid),
                lambda: nc.vector.tensor_tensor(
                    out=gdu[:], in0=ps_g,
                    in1=s1gu_b[:].rearrange("p t n -> p (t n)"), op=mult),
                (lambda: nc.vector.tensor_tensor(
                    out=t1[:], in0=gdu[:], in1=sig[:], op=mult)) if last else
                (lambda: nc.gpsimd.tensor_tensor(
                    out=t1[:], in0=gdu[:], in1=sig[:], op=mult)),
                lambda: nc.vector.tensor_tensor(
                    out=act2[:], in0=t1[:], in1=ps_u, op=mult),
                lambda: nc.gpsimd.partition_all_reduce(
                    am[:], act2[:], channels=128,
                    reduce_op=bass_isa.ReduceOp.absmax),
                lambda: nc.vector.tensor_reduce(
                    out=m[:], in_=am[:].rearrange("p (t n) -> p n t", t=FT),
                    op=mybir.AluOpType.max, axis=mybir.AxisListType.X),
                lambda: nc.vector.tensor_scalar_max(
                    out=mc[:], in0=m[:], scalar1=1e-12),
                lambda: nc.vector.reciprocal(out=r[:], in_=mc[:]),
            ]
            tqv = tq[:].rearrange("p (t n) -> p t n", t=FT)
            tq2v = tq2[:].rearrange("p (t n) -> p t n", t=FT)
            a2v = act2[:].rearrange("p (t n) -> p t n", t=FT)
            if not last:
                ops += [
                    lambda: nc.vector.scalar_tensor_tensor(
                        out=tqv, in0=a2v, scalar=127.0,
                        in1=r[:].unsqueeze(1).broadcast_to([128, FT, n_pad]),
                        op0=mult, op1=mult),
                    lambda: nc.scalar.activation(
                        out=tq2[:], in_=tq[:],
                        func=mybir.ActivationFunctionType.Copy, bias=MAGIC),
                    lambda: nc.scalar.activation(
                        out=aq[:].rearrange("p t n -> p (t n)"), in_=tq2[:],
                        func=mybir.ActivationFunctionType.Copy, bias=-MAGIC),
                ]
            else:
                # per-GEMM2-k-tile requant: aq chunk ki is ready ~3 small ops
                # after r, so GEMM2 can start immediately
                for ki in range(KI):
                    ops += [
                        (lambda ki=ki: nc.vector.scalar_tensor_tensor(
                            out=tqv[:, ki : ki + 1, :],
                            in0=a2v[:, ki : ki + 1, :], scalar=127.0,
                            in1=r[:].unsqueeze(1).broadcast_to([128, 1, n_pad]),
                            op0=mult, op1=mult)),
                        (lambda ki=ki: nc.scalar.activation(
                            out=tq2v[:, ki : ki + 1, :], in_=tqv[:, ki : ki + 1, :],
                            func=mybir.ActivationFunctionType.Copy, bias=MAGIC)),
                        (lambda ki=ki: nc.vector.tensor_scalar_add(
                            out=aq[:, ki : ki + 1, :], in0=tq2v[:, ki : ki + 1, :],
                            scalar1=-MAGIC)),
                    ]
            ops += [
                lambda: nc.vector.scalar_tensor_tensor(
                    out=s2c[:], in0=mc[:], scalar=1.0 / 127.0,
                    in1=comb_s[:, e, :], op0=mult, op1=mult),
                lambda: nc.gpsimd.tensor_tensor(
                    out=w2sc[:],
                    in0=w2s_s[:, e, :].unsqueeze(2).broadcast_to([128, HT, n_pad]),
                    in1=s2c[:].unsqueeze(1).broadcast_to([128, HT, n_pad]),
                    op=mult),
            ]
            return ops

        def gemm2(e, aq, w2sc, w2tiles):
            """weight-stationary GEMM2 + per-bank dequant + output DMA.
            PSUM can only be read once a bank's accumulation group stopped,
            so the last-ki MMs and the dequant proceed bank by bank."""
            ps2 = ps2_pool.tile([128, HW], F32, tag="ps2", name=f"ps2_{e}")
            o_sb = op_.tile([128, HW], BF16, tag="o_sb", name=f"o_{e}")
            cpb = max(1, 2048 // (n_pad * 4))  # h-chunks per PSUM bank
            banks = [list(range(b, min(b + cpb, HT))) for b in range(0, HT, cpb)]
            final_ki = KI - 1
            dve_tail = e == EPC - 1
            for ki in range(KI):
                last = ki == final_ki
                for bi, bchunks in enumerate(banks):
                    for t in bchunks:
                        if last and dve_tail:
                            nb2 = HT // 2
                            lhsT = w2half[t // nb2][:, (t % nb2) * 128 : (t % nb2 + 1) * 128]
                        else:
                            lhsT = w2tiles[ki][:, t * 128 : (t + 1) * 128]
                        nc.tensor.matmul(
                            ps2[:, t * n_pad : (t + 1) * n_pad],
                            lhsT=lhsT,
                            rhs=aq[:, ki, :],
                            start=(ki == 0 and t in g2_starts),
                            stop=(last and t in g2_stops),
                        )
                    if not last:
                        continue
                    lo = bchunks[0] * n_pad
                    hi = (bchunks[-1] + 1) * n_pad
                    if bi < len(banks) - 1 and not dve_tail:
                        # earlier banks: ACT copies psum out, GPSIMD scales
                        od = epi.tile([128, hi - lo], F32, tag="odeq", name=f"od_{e}_{bi}")
                        nc.scalar.activation(
                            out=od[:], in_=ps2[:, lo:hi],
                            func=mybir.ActivationFunctionType.Copy,
                        )
                        nc.gpsimd.tensor_tensor(
                            out=o_sb[:, lo:hi],
                            in0=od[:],
                            in1=w2sc[:, bchunks[0] : bchunks[-1] + 1, :].rearrange(
                                "p t n -> p (t n)"
                            ),
                            op=mult,
                        )
                    else:
                        # final bank: one DVE op for the shortest tail chain
                        nc.vector.tensor_tensor(
                            out=o_sb[:, lo:hi],
                            in0=ps2[:, lo:hi],
                            in1=w2sc[:, bchunks[0] : bchunks[-1] + 1, :].rearrange(
                                "p t n -> p (t n)"
                            ),
                            op=mult,
                        )
                    nc.scalar.dma_start(
                        out=o_d[e, :, lo:hi], in_=o_sb[:, lo:hi]
                    )

        # --- main pipeline: expert-sequential so epilogue(e0) hides under
        # --- expert 1's cast stream; w2(e1) is the last DMA (tail-paced)
        g1_chunks = [(h, t) for h in (0, 1) for t in range(FT)]
        g1_chunks_last = [(h, t) for h in (1, 0) for t in range(FT)]
        s_idx, _ = bank_flags([h * FW * 4 + t * n_pad * 4 for h, t in g1_chunks])
        _, e_idx = bank_flags([h * FW * 4 + t * n_pad * 4 for h, t in g1_chunks_last])
        g1_starts = {g1_chunks[i] for i in s_idx}
        g1_stops = {g1_chunks_last[i] for i in e_idx}
        g2_starts, g2_stops = bank_flags([t * n_pad * 4 for t in range(HT)])
        w2tiles = {}
        epi_res = {e: {} for e in range(EPC)}
        pending = []
        for e in range(EPC):
            ps1_e = ps1_pool.tile([128, 2 * FW], F32, tag="ps1", name=f"ps1_{e}")
            for k in range(KH):
                w1i8 = w1i8p.tile([128, I2], I8, tag="w1i8", name=f"w1i8_{e}_{k}")
                nc.sync.dma_start(out=w1i8[:], in_=w1_d[e, k])
                w1f = w1f16p.tile([128, I2], F16, tag="w1f", name=f"w1f_{e}_{k}")
                o0, n0 = CAST_DVE
                nc.vector.tensor_copy(out=w1f[:, o0 : o0 + n0], in_=w1i8[:, o0 : o0 + n0])
                o1, n1 = CAST_ACT
                nc.scalar.activation(
                    out=w1f[:, o1 : o1 + n1],
                    in_=w1i8[:, o1 : o1 + n1],
                    func=mybir.ActivationFunctionType.Copy,
                )
                o2, n2 = CAST_POOL
                nc.gpsimd.tensor_copy(out=w1f[:, o2 : o2 + n2], in_=w1i8[:, o2 : o2 + n2])
                rhs = xqs_s[:, e, k, :]
                chunks = g1_chunks if k < KH - 1 else g1_chunks_last
                for half, t in chunks:
                    base = half * FW
                    nc.tensor.matmul(
                        ps1_e[:, base + t * n_pad : base + (t + 1) * n_pad],
                        lhsT=w1f[:, half * I + t * 128 : half * I + (t + 1) * 128],
                        rhs=rhs,
                        start=(k == 0 and (half, t) in g1_starts),
                        stop=(k == KH - 1 and (half, t) in g1_stops),
                    )
                # sprinkle the previous expert's epilogue into this cast
                # stream so its cross-engine chain never stalls the casts
                if pending and k >= 1:
                    pending.pop(0)()
            while pending:
                pending.pop(0)()
            pending = epilogue1_ops(e, ps1_e, epi_res[e], last=(e == EPC - 1))
        # all w2 DMAs ride behind the full w1 stream in the SP queue; the
        # very last k-tile (last expert) ships as two half-width tiles
        w2half = {}
        for e in range(EPC):
            w2tiles[e] = []
            nk = KI - 1 if e == EPC - 1 else KI
            for ki in range(nk):
                w2t = w2p.tile([128, H], F8E3, tag=f"w2_{e}_{ki}", name=f"w2_{e}_{ki}")
                nc.sync.dma_start(out=w2t[:], in_=w2_d[e, ki])
                w2tiles[e].append(w2t)
        e = EPC - 1
        ha = w2p.tile([128, H // 2], F8E3, tag="w2ha", name="w2_last_a")
        nc.sync.dma_start(out=ha[:], in_=w2_d[e, KI - 1][:, 0 : H // 2])
        hb = w2p.tile([128, H // 2], F8E3, tag="w2hb", name="w2_last_b")
        nc.sync.dma_start(out=hb[:], in_=w2_d[e, KI - 1][:, H // 2 : H])
        w2half = {0: ha, 1: hb}
        # last expert's epilogue emitted inline
        while pending:
            pending.pop(0)()
        for e in range(EPC):
            gemm2(e, epi_res[e]["aq"], epi_res[e]["w2sc"], w2tiles[e])

    nc.compile()
    return nc


def get_program(n_pad: int):
    key = ("nc", n_pad)
    if key not in _cache:
        _cache[key] = _build_program(n_pad)
    return _cache[key]


def _routing(expert_ids, expert_scales):
    """comb[B, E] scatter-add; token lists per expert; N_PAD."""
    comb = np.zeros((B, E), np.float32)
    np.add.at(comb, (np.arange(B)[:, None], np.asarray(expert_ids)),
              np.asarray(expert_scales, np.float32))
    routed = np.zeros((B, E), bool)
    routed[np.arange(B)[:, None], np.asarray(expert_ids)] = True
    toks = [np.nonzero(routed[:, e])[0] for e in range(E)]
    max_n = max(len(t) for t in toks)
    n_pad = 16
    while n_pad < max_n:
        n_pad *= 2
    # PSUM chunking requires pow2 n_pad; >64 would overflow the 8 banks
    assert n_pad <= 64, f"routing too dense for this kernel: n_pad={n_pad}"
    return comb, toks, n_pad


def _prep_inputs(x, expert_ids, smooth_scales, expert_scales, w1, w1_scale, w2, w2_scale):
    """Host-side dispatch: quantize x, route tokens, shard experts."""
    x = np.asarray(x, np.float32)
    smooth_scales = np.asarray(smooth_scales, np.float32)
    w1_scale = np.asarray(w1_scale, np.float32)
    w2_scale = np.asarray(w2_scale, np.float32)

    # dynamic per-token int8 quantization (exact mirror of reference ops)
    sx = np.maximum(np.max(np.abs(x), axis=-1, keepdims=True), 1e-12) / 127.0
    xq = np.round(np.clip(x / sx, -128.0, 127.0)).astype(np.float32)
    xqs = (xq * sx).astype(np.float16)  # [B, H]
    xqsT = np.ascontiguousarray(
        xqs.T.reshape(KH, 128, B).transpose(1, 0, 2)
    )  # [128, KH, B]

    comb, toks, n_pad = _routing(expert_ids, expert_scales)

    w1v = np.asarray(w1).astype(np.int8)
    w2v = np.asarray(w2).astype(np.int8)

    in_maps = []
    for c in range(NCORES):
        es = list(range(c * EPC, (c + 1) * EPC))
        xqsT_e = np.zeros((EPC, 128, KH, n_pad), np.float16)
        comb_e = np.zeros((EPC, 128, n_pad), np.float32)
        for i, e in enumerate(es):
            tk = toks[e]
            xqsT_e[i, :, :, : len(tk)] = xqsT[:, :, tk]
            comb_e[i, :, : len(tk)] = comb[tk, e][None, :]
        w1c = w1v[es].reshape(EPC, KH, 128, I2)
        w2c = np.ascontiguousarray(
            (w2v[es].reshape(EPC, KI, 128, H).astype(np.float32) / 16.0)
        ).astype(ml_dtypes.float8_e3m4)
        # per-partition scale columns [e, p, T]
        s1g_full = w1_scale[es][:, :I]
        s1u_full = w1_scale[es][:, I:] * smooth_scales[es]
        s1g = np.ascontiguousarray(s1g_full.reshape(EPC, FT, 128).transpose(0, 2, 1))
        s1gu = np.ascontiguousarray(
            (s1g_full * s1u_full).reshape(EPC, FT, 128).transpose(0, 2, 1))
        sc2 = np.ascontiguousarray(
            (w2_scale[es] * 16.0).reshape(EPC, HT, 128).transpose(0, 2, 1))
        in_maps.append(
            {
                "xqsT": xqsT_e,
                "w1t": np.ascontiguousarray(w1c),
                "w2t": w2c,
                "s1g": s1g.astype(np.float32),
                "s1gu": s1gu.astype(np.float32),
                "scale2": sc2.astype(np.float32),
                "combs": comb_e,
            }
        )
    return in_maps, toks, n_pad


def kernel(
    x,
    expert_ids,
    smooth_scales,
    expert_scales,
    x_active_mask,
    w1,
    w1_scale,
    w2,
    w2_scale,
    _trace=False,
    _trace_kwargs=None,
):
    in_maps, toks, n_pad = _prep_inputs(
        x, expert_ids, smooth_scales, expert_scales, w1, w1_scale, w2, w2_scale
    )
    nc = get_program(n_pad)
    res = run_bass_kernel_spmd(
        nc,
        in_maps,
        core_ids=list(range(NCORES)),
        trace=_trace,
        **(_trace_kwargs or {}),
    )
    y = np.zeros((B, H), np.float32)
    for c, r in enumerate(res.results):
        o = np.asarray(r["o"], np.float32).reshape(EPC, 128, HT, n_pad)
        for i in range(EPC):
            e = c * EPC + i
            tk = toks[e]
            contrib = o[i, :, :, : len(tk)].transpose(2, 1, 0).reshape(len(tk), H)
            y[tk] += contrib
    y *= np.asarray(x_active_mask).astype(np.float32)[:, None]
    if _trace:
        kernel.last_results = res
    return y
